# revision 1
# baseline (speedup 1.0000x reference)
"""Multi-head attention (B=2, S=2048, E=2048, H=16, causal) on 8 TRN2 NeuronCores.

Sharding: 8 cores = 2 batch shards x 4 head-group shards (4 heads / 512
features each).  Each core runs the full attention stack for its (batch,
head-group) and produces a partial [S, E] output through its row-block of
Wo; the host sums the 4 partials per batch.

Projections (QKV, Wo) run as 3-term fp8e4m3 hi/lo split matmuls in
DoubleRow perf mode (A@B ~ A1B1 + A2B1 + A1B2, each term contracting
2x128 rows per pass at 0.5 cyc/row).  The hi/lo splits of x and the
weights are prepared on the host; weights are pre-scaled by 64 so their
values sit in fp8's normal range (the scale is undone on the way out).
Attention (scores, attn@v, rowsum) runs in fp16 at full PE rate.
"""

import os

import numpy as np

import concourse.bacc as bacc
import concourse.mybir as mybir
import concourse.tile as tile
from concourse import bass_utils

_T = {
    "EPOOL": int(os.environ.get("K_EPOOL", "10")),
    "PEND": int(os.environ.get("K_PEND", "3")),
    "RESV": int(os.environ.get("K_RESV", "2")),
    "EMOD": int(os.environ.get("K_EMOD", "12")),
    "EPH": int(os.environ.get("K_EPH", "3")),
    "PSS": int(os.environ.get("K_PSS", "4")),
    "PSR": int(os.environ.get("K_PSR", "1")),
}

B, S, E, H = 2, 2048, 2048, 16
D = 128                    # head dim
HL = 4                     # heads per core
F = HL * D                 # local features = 512
EO = E // 128              # 16 contraction chunks
TT = 256                   # phase-1 token tile
IT = 512                   # phase-2 query tile
F32 = mybir.dt.float32
F16 = mybir.dt.float16
F8 = mybir.dt.float8e4
DR = mybir.MatmulPerfMode.DoubleRow
EXP = mybir.ActivationFunctionType.Exp
COPY = mybir.ActivationFunctionType.Copy
WS = 64.0                  # host-side weight prescale
SCALE = 1.0 / float(np.sqrt(D)) / (WS * WS)
NPF8 = mybir.dt.np(F8)

_CACHE = {}


def _build():
    nc = bacc.Bacc("TRN2", target_bir_lowering=False, debug=False)
    # x packed on host as [ei=128, tt, eo, TT] so each phase-1 tile loads as
    # one DMA with 4KB-contiguous per-partition payload (full bus efficiency)
    x1P = nc.dram_tensor("x1P", [128, S // TT, EO, TT], F8,
                         kind="ExternalInput").ap()
    x2P = nc.dram_tensor("x2P", [128, S // TT, EO, TT], F8,
                         kind="ExternalInput").ap()
    # all weights pre-packed on host into their SBUF layouts (contiguous
    # per-partition payloads -> full DMA bus efficiency); q/k are fc-major
    # so the first head-column can land in one small DMA at startup
    w_t = {
        n: nc.dram_tensor(n, [128, HL, EO, 128], F8, kind="ExternalInput").ap()
        for n in ("wq1", "wq2", "wk1", "wk2")
    }
    for n in ("wv1", "wv2"):
        w_t[n] = nc.dram_tensor(n, [128, EO, F], F8, kind="ExternalInput").ap()
    wo1_t = nc.dram_tensor("wo1", [128, HL, E], F8, kind="ExternalInput").ap()
    wo2_t = nc.dram_tensor("wo2", [128, HL, E], F8, kind="ExternalInput").ap()
    # [TRI | 100*I]: one PE matmul accumulates -6e6 onto the strictly-upper
    # triangle of a diagonal scores block (additive causal mask; exp -> 0)
    trieye = nc.dram_tensor("trieye", [128, 256], F16, kind="ExternalInput").ap()
    y = nc.dram_tensor("y", [S, E], mybir.dt.bfloat16, kind="ExternalOutput").ap()

    with tile.TileContext(nc) as tc:
        with tc.tile_pool(name="persist", bufs=1) as persist:
            qT = persist.tile([128, HL, S], F16, tag="qT")
            kT = persist.tile([128, HL, S], F16, tag="kT")
            vN = persist.tile([128, S // 128, F], F16, tag="vN")
            out1 = persist.tile([128, HL, S], F8, tag="out1")
            out2 = persist.tile([128, HL, S], F8, tag="out2")
            triT = persist.tile([128, 256], F16, tag="triT")
            onesT_f = persist.tile([128, 1], F32, tag="onesT_f")
            onesT = persist.tile([128, 1], F16, tag="onesT")
            onesT32 = persist.tile([128, 1], mybir.dt.float32r, tag="onesT32")

            # rowsum weights = WS so that rec = 1/(WS * sum(exp)) folds the
            # v-side prescale away in the normalize multiply
            nc.vector.memset(onesT_f[:], WS)
            nc.vector.tensor_copy(onesT[:], onesT_f[:])
            nc.vector.tensor_copy(onesT32[:], onesT_f[:])

            # ---------- phase 1: q/k/v projections (single pass) ----------
            with (
                tc.tile_pool(name="wres", bufs=1) as wpool,
                tc.tile_pool(name="xstream", bufs=4) as xpool,
                tc.tile_pool(name="ps_qk", bufs=5, space="PSUM") as ps_qk,
                tc.tile_pool(name="ps_v", bufs=2, space="PSUM") as ps_v,
            ):
                wres = {}
                for n in ("wq1", "wq2", "wk1", "wk2"):
                    wres[n] = wpool.tile([128, HL, EO, 128], F8, tag=n, name=n)
                for n in ("wv1", "wv2"):
                    wres[n] = wpool.tile([128, EO, F], F8, tag=n, name=n)
                # startup order: first x tile + wq first (few large DMAs),
                # matching the order phase-1 matmul chains consume them
                xt = {}
                xt[0] = (xpool.tile([128, EO, TT], F8, tag="x1", name="x1a"),
                         xpool.tile([128, EO, TT], F8, tag="x2", name="x2a"))
                xt[1] = (xpool.tile([128, EO, TT], F8, tag="x1", name="x1b"),
                         xpool.tile([128, EO, TT], F8, tag="x2", name="x2b"))
                # first loads fan out over three issue queues so the SEQ/DGE
                # lead-in overlaps; transfers still serialize on the DMA bus
                nc.sync.dma_start(wres["wq1"][:, 0], w_t["wq1"][:, 0])
                nc.sync.dma_start(xt[0][0][:, 0:8], x1P[:, 0, 0:8])
                nc.sync.dma_start(xt[0][0][:, 8:], x1P[:, 0, 8:])
                nc.sync.dma_start(xt[0][1][:, 0:8], x2P[:, 0, 0:8])
                nc.sync.dma_start(xt[0][1][:, 8:], x2P[:, 0, 8:])
                nc.sync.dma_start(wres["wq2"][:, 0], w_t["wq2"][:, 0])
                nc.sync.dma_start(wres["wq1"][:, 1], w_t["wq1"][:, 1])
                nc.sync.dma_start(wres["wq2"][:, 1], w_t["wq2"][:, 1])
                nc.sync.dma_start(wres["wq1"][:, 2], w_t["wq1"][:, 2])
                nc.sync.dma_start(wres["wq2"][:, 2], w_t["wq2"][:, 2])
                nc.sync.dma_start(wres["wq1"][:, 3], w_t["wq1"][:, 3])
                nc.sync.dma_start(wres["wq2"][:, 3], w_t["wq2"][:, 3])
                nc.sync.dma_start(wres["wk1"][:, 0], w_t["wk1"][:, 0])
                nc.sync.dma_start(wres["wk2"][:, 0], w_t["wk2"][:, 0])
                nc.sync.dma_start(wres["wk1"][:, 1], w_t["wk1"][:, 1])
                nc.sync.dma_start(wres["wk2"][:, 1], w_t["wk2"][:, 1])
                nc.sync.dma_start(wres["wk1"][:, 2], w_t["wk1"][:, 2])
                nc.sync.dma_start(wres["wk2"][:, 2], w_t["wk2"][:, 2])
                nc.sync.dma_start(wres["wk1"][:, 3], w_t["wk1"][:, 3])
                nc.sync.dma_start(wres["wk2"][:, 3], w_t["wk2"][:, 3])
                nc.sync.dma_start(xt[1][0][:], x1P[:, 1])
                nc.sync.dma_start(xt[1][1][:], x2P[:, 1])
                for n in ("wv1", "wv2"):
                    nc.sync.dma_start(wres[n][:], w_t[n])
                for tt in range(S // TT):
                    t0 = tt * TT
                    if tt in xt:
                        x1, x2 = xt.pop(tt)
                    else:
                        x1 = xpool.tile([128, EO, TT], F8, tag="x1")
                        x2 = xpool.tile([128, EO, TT], F8, tag="x2")
                        nc.sync.dma_start(x1[:], x1P[:, tt])
                        nc.sync.dma_start(x2[:], x2P[:, tt])
                    if tt == 4:
                        nc.sync.dma_start(triT[:], trieye)
                    for wn, dst in (("wq", qT), ("wk", kT)):
                        w1, w2 = wres[wn + "1"], wres[wn + "2"]
                        for fc in range(HL):
                            ps = ps_qk.tile([128, TT], F32, tag="pqk")
                            terms = (
                                [(w1, x1, g) for g in range(0, EO, 2)]
                                + [(w1, x2, g) for g in range(0, EO, 2)]
                                + [(w2, x1, g) for g in range(0, EO, 2)]
                            )
                            for i, (w, x, g) in enumerate(terms):
                                nc.tensor.matmul(
                                    ps[:],
                                    w[:, fc, g:g + 2, :],
                                    x[:, g:g + 2, :],
                                    start=(i == 0),
                                    stop=(i == len(terms) - 1),
                                    perf_mode=DR,
                                )
                            nc.vector.tensor_copy(dst[:, fc, t0:t0 + TT], ps[:])
                    w1, w2 = wres["wv1"], wres["wv2"]
                    for tc2 in range(TT // 128):
                        tsl = slice(tc2 * 128, (tc2 + 1) * 128)
                        ps = ps_v.tile([128, F], F32, tag="pv")
                        terms = (
                            [(x1, w1, g) for g in range(0, EO, 2)]
                            + [(x2, w1, g) for g in range(0, EO, 2)]
                            + [(x1, w2, g) for g in range(0, EO, 2)]
                        )
                        for i, (x, w, g) in enumerate(terms):
                            nc.tensor.matmul(
                                ps[:],
                                x[:, g:g + 2, tsl],
                                w[:, g:g + 2, :],
                                start=(i == 0),
                                stop=(i == len(terms) - 1),
                                perf_mode=DR,
                            )
                        nc.vector.tensor_copy(
                            vN[:, (t0 // 128) + tc2, :], ps[:]
                        )

            # ---------- phase 2: attention per head ----------------------
            with tc.tile_pool(name="wo", bufs=1) as wo_pool:
                wo1_r = wo_pool.tile([128, HL, E], F8, tag="wo1")
                wo2_r = wo_pool.tile([128, HL, E], F8, tag="wo2")
                nc.sync.dma_start(wo1_r[:], wo1_t)
                nc.sync.dma_start(wo2_r[:], wo2_t)

                with (
                    tc.tile_pool(name="ph2", bufs=_T["EPOOL"]) as epool,
                    tc.tile_pool(name="ph2s", bufs=3) as spool,
                    tc.tile_pool(name="ph2b", bufs=3) as small,
                    tc.tile_pool(name="ph2f", bufs=3) as fpool,
                    tc.tile_pool(name="ps_s", bufs=_T["PSS"], space="PSUM") as ps_s,
                    tc.tile_pool(name="ps_o", bufs=2, space="PSUM") as ps_o,
                    tc.tile_pool(name="ps_r", bufs=_T["PSR"], space="PSUM") as ps_r,
                    tc.tile_pool(name="ps_yb", bufs=1, space="PSUM") as ps_yb,
                    tc.tile_pool(name="ystb", bufs=4) as ystb_pool,
                ):
                    ready_y = []

                    def emit_y_pair(split_dma=False, in_ph2=True):
                        # one (tcb, et-pair) group: two Wo psum chains into a
                        # single [128, 1024] bf16 store
                        tcb, ep = ready_y.pop(0)
                        tsl = slice(tcb * 128, (tcb + 1) * 128)
                        yb = ystb_pool.tile([128, 1024], mybir.dt.bfloat16,
                                            tag="yb")
                        for j in range(2):
                            esl = slice((2 * ep + j) * 512,
                                        (2 * ep + j + 1) * 512)
                            if j == 0 and in_ph2:
                                Yb = ps_yb.tile([128, 512], F32, tag="Yb")
                            else:
                                # borrow a scores-pool buffer (same tag so no
                                # extra PSUM banks are reserved); in phase 3
                                # all chains rotate the 3 scores buffers
                                Yb = ps_s.tile([128, IT], F32, tag="S")
                            terms = []
                            for fp in range(HL // 2):
                                g = 2 * fp
                                terms += [(out1, wo1_r, g), (out2, wo1_r, g),
                                          (out1, wo2_r, g)]
                            for i, (o, w, g) in enumerate(terms):
                                nc.tensor.matmul(
                                    Yb[:],
                                    o[:, g:g + 2, tsl],
                                    w[:, g:g + 2, esl],
                                    start=(i == 0),
                                    stop=(i == len(terms) - 1),
                                    perf_mode=DR,
                                )
                            if j == 0:
                                nc.vector.tensor_scalar_mul(
                                    yb[:, 0:512], Yb[:], 1.0 / WS
                                )
                            elif in_ph2:
                                # DVE (gpsimd cannot read PSUM; Act is on exp)
                                nc.vector.tensor_scalar_mul(
                                    yb[:, 512:1024], Yb[:], 1.0 / WS
                                )
                            else:
                                nc.scalar.activation(
                                    yb[:, 512:1024], Yb[:], COPY,
                                    scale=1.0 / WS,
                                )
                            if split_dma:
                                nc.sync.dma_start(y[tsl, esl],
                                                  yb[:, j * 512:(j + 1) * 512])
                        if not split_dma:
                            nc.sync.dma_start(
                                y[tsl, ep * 1024:(ep + 1) * 1024], yb[:]
                            )

                    deferred_split = []

                    def flush_split():
                        while deferred_split:
                            Ocp_d, h_d, i0_d = deferred_split.pop(0)
                            # hi-copy on Pool: Act stays dedicated to exp
                            nc.gpsimd.tensor_copy(
                                out1[:, h_d, i0_d:i0_d + IT], Ocp_d[:]
                            )
                            nc.vector.tensor_sub(
                                out2[:, h_d, i0_d:i0_d + IT], Ocp_d[:],
                                out1[:, h_d, i0_d:i0_d + IT],
                            )
                            if h_d == HL - 1:
                                # the head-group's outputs are now all
                                # written -- its y pairs may be emitted
                                pd = i0_d // IT
                                for tcb_r in range(4 * pd, 4 * pd + 4):
                                    for ep_r in range(E // 1024):
                                        ready_y.append((tcb_r, ep_r))

                    # p0 (shortest, exp-overhead-heavy) runs last, when
                    # y-pair chains exist to fill PE while Act drains
                    p_order = (1, 2, 3, 0)
                    for pi, p in enumerate(p_order):
                        i0 = p * IT
                        for h in range(HL):
                            h0 = h * 128
                            njc = (i0 + IT) // 128
                            O = ps_o.tile([128, IT], F32, tag="O")
                            R = ps_r.tile([1, IT], F32, tag="R")

                            def emit_scores(jc):
                                q_off = jc - (i0 // 128)
                                # diag chunk q: columns i < 128*q are fully
                                # masked -- compute only the valid slice
                                off = 0 if q_off < 0 else 128 * q_off
                                diag = q_off >= 0
                                Sps = ps_s.tile([128, IT], F32, tag="S")
                                nc.tensor.matmul(
                                    Sps[:, off:],
                                    kT[:, h, jc * 128:(jc + 1) * 128],
                                    qT[:, h, i0 + off:i0 + IT],
                                    start=True,
                                    stop=not diag,
                                )
                                if diag:
                                    # additive causal mask on the diagonal
                                    # 128-block via one tiny matmul
                                    nc.tensor.matmul(
                                        Sps[:, off:off + 128],
                                        triT[:, 0:128],
                                        triT[:, 128:256],
                                        start=False,
                                        stop=True,
                                    )
                                Et = epool.tile([128, IT], F16, tag="E")
                                nc.scalar.activation(
                                    Et[:, off:], Sps[:, off:], EXP,
                                    scale=SCALE,
                                )
                                return Et, off

                            r_st = {"on": False}

                            def emit_av(jc, Et, off):
                                nc.tensor.matmul(
                                    O[:, off:],
                                    vN[:, jc, h0:h0 + 128],
                                    Et[:, off:],
                                    start=(jc == 0),
                                    stop=(jc == njc - 1),
                                )
                                if jc >= i0 // 128:
                                    # diag chunks: per-chunk rowsum
                                    nc.tensor.matmul(
                                        R[:, off:],
                                        onesT[:],
                                        Et[:, off:],
                                        start=not r_st["on"],
                                        stop=(jc == njc - 1),
                                    )
                                    r_st["on"] = True

                            def emit_rquad(EtQ):
                                # one PE rowsum covers FOUR chunks, pre-summed
                                # on DVE (f32r quad sums avoid fp16 overflow)
                                nc.tensor.matmul(
                                    R[:],
                                    onesT32[:],
                                    EtQ[:],
                                    start=not r_st["on"],
                                    stop=False,
                                    skip_group_check=True,
                                )
                                r_st["on"] = True

                            # scores/exp run a few chunks ahead of attn@v
                            pending = []
                            rquads = []
                            prev_et = None
                            prev_es = None
                            for jc in range(njc):
                                # y-pair chains at the iteration start give
                                # the Act engine room to drain the previous
                                # iteration's exp backlog (keep a reserve to
                                # cover the final normalize gap too)
                                if (jc % _T["EMOD"] == _T["EPH"]
                                        and len(ready_y) > _T["RESV"]):
                                    emit_y_pair()
                                Et, off = emit_scores(jc)
                                pending.append((jc, Et, off))
                                if jc < i0 // 128:
                                    if jc % 2 == 1:
                                        EtS = spool.tile([128, IT], F16,
                                                         tag="EtS")
                                        nc.vector.tensor_add(
                                            EtS[:], prev_et[:], Et[:]
                                        )
                                        if jc % 4 == 3:
                                            EtQ = spool.tile(
                                                [128, IT],
                                                mybir.dt.float32r, tag="EtQ")
                                            nc.vector.tensor_add(
                                                EtQ[:], prev_es[:], EtS[:]
                                            )
                                            rquads.append(EtQ)
                                        prev_es = EtS
                                    prev_et = Et
                                else:
                                    # diag region: flush pending quad sums
                                    while rquads:
                                        emit_rquad(rquads.pop(0))
                                if len(rquads) > 1:
                                    emit_rquad(rquads.pop(0))
                                if len(pending) > _T["PEND"]:
                                    emit_av(*pending.pop(0))
                            for item in pending:
                                emit_av(*item)
                            rec = small.tile([1, IT], F32, tag="rec")
                            nc.vector.reciprocal(rec[:], R[:])
                            RB = small.tile([128, IT], F32, tag="RB")
                            nc.gpsimd.partition_broadcast(RB[:], rec[:])
                            Ocp = fpool.tile([128, IT], F32, tag="Ocp")
                            nc.vector.tensor_mul(Ocp[:], O[:], RB[:])
                            # defer the fp8 hi/lo split of this iteration's
                            # output until after the next iteration's
                            # broadcast, so Pool's broadcast is never queued
                            # behind a hi-copy
                            flush_split()
                            deferred_split.append((Ocp, h, i0))
                            if pi == len(p_order) - 1 and h == HL - 1:
                                flush_split()

                    # ---- phase 3: remaining output-projection groups ------
                    while ready_y:
                        emit_y_pair(split_dma=False,
                                    in_ph2=False)
    nc.compile()
    return nc


def _get_nc():
    if "nc" not in _CACHE:
        _CACHE["nc"] = _build()
    return _CACHE["nc"]


def _split8(a):
    hi = a.astype(NPF8)
    lo = (a - hi.astype(np.float32)).astype(NPF8)
    return hi, lo


def make_in_maps(x, Wq, Wk, Wv, Wo):
    x = np.asarray(x, np.float32)
    Wq = np.asarray(Wq, np.float32)
    Wk = np.asarray(Wk, np.float32)
    Wv = np.asarray(Wv, np.float32)
    Wo = np.asarray(Wo, np.float32)

    trieye = np.zeros((128, 256), np.float16)
    trieye[:, 0:128] = np.triu(np.full((128, 128), -60000.0), k=1)
    trieye[:, 128:256] = 100.0 * np.eye(128)
    trieye = np.ascontiguousarray(trieye)

    def _pack_x(xT8):
        # [E, S] -> [ei=128, tt, eo, TT] (phase-1 tile DMA order)
        return np.ascontiguousarray(
            xT8.reshape(EO, 128, S // TT, TT).transpose(1, 2, 0, 3)
        )

    xs = []
    for b in range(B):
        hi, lo = _split8(np.ascontiguousarray(x[b].T))
        xs.append((_pack_x(hi), _pack_x(lo)))
    def _pack_qk(w8):  # [E, F] -> [ei=128, fc, eo, 128]
        return np.ascontiguousarray(
            w8.reshape(EO, 128, HL, 128).transpose(1, 2, 0, 3)
        )

    def _pack_v(w8):  # [E, F] -> [ei=128, eo, F]
        return np.ascontiguousarray(
            w8.reshape(EO, 128, F).transpose(1, 0, 2)
        )

    def _pack_o(w8):  # [F, E] -> [fi=128, fc, E]
        return np.ascontiguousarray(
            w8.reshape(HL, 128, E).transpose(1, 0, 2)
        )

    in_maps = []
    for c in range(8):
        b, g = c // 4, c % 4
        fsl = slice(F * g, F * (g + 1))
        m = {"x1P": xs[b][0], "x2P": xs[b][1], "trieye": trieye}
        for n, W, pk in (("wq", Wq, _pack_qk), ("wk", Wk, _pack_qk),
                         ("wv", Wv, _pack_v)):
            hi, lo = _split8(np.ascontiguousarray(W[fsl, :].T) * WS)
            m[n + "1"], m[n + "2"] = pk(hi), pk(lo)
        hi, lo = _split8(np.ascontiguousarray(Wo[:, fsl].T) * WS)
        m["wo1"], m["wo2"] = _pack_o(hi), _pack_o(lo)
        in_maps.append(m)
    return in_maps


def combine_outputs(results):
    out = np.empty((B, S, E), np.float32)
    for b in range(B):
        acc = results[4 * b]["y"].astype(np.float32).copy()
        for g in range(1, 4):
            acc += results[4 * b + g]["y"]
        out[b] = acc
    return out


def kernel(x, Wq, Wk, Wv, Wo):
    import time as _time

    nc = _get_nc()
    in_maps = make_in_maps(x, Wq, Wk, Wv, Wo)
    last_exc = None
    for attempt in range(3):
        if attempt:
            # transient device wedge (e.g. NRT_EXEC_UNIT_UNRECOVERABLE) --
            # wait for recovery before retrying
            _time.sleep(30 * attempt)
        try:
            res = bass_utils.run_bass_kernel_spmd(
                nc, in_maps, core_ids=list(range(8))
            )
            return combine_outputs(res.results)
        except Exception as exc:
            last_exc = exc
    raise last_exc



# revision 60
# speedup vs baseline: 1.0405x; 1.0405x over previous
"""Multi-head attention (B=2, S=2048, E=2048, H=16, causal) on 8 TRN2 NeuronCores.

Sharding: 8 cores = 2 batch shards x 4 head-group shards (4 heads / 512
features each).  Each core runs the full attention stack for its (batch,
head-group) and produces a partial [S, E] output through its row-block of
Wo; the host sums the 4 partials per batch.

Projections (QKV, Wo) run as 3-term fp8e4m3 hi/lo split matmuls in
DoubleRow perf mode (A@B ~ A1B1 + A2B1 + A1B2, each term contracting
2x128 rows per pass at 0.5 cyc/row).  The hi/lo splits of x and the
weights are prepared on the host; weights are pre-scaled by 64 so their
values sit in fp8's normal range (the scale is undone on the way out).
Attention (scores, attn@v) runs in fp16 at full PE rate.

Scheduling structure:
- softmax rowsum and causal mask run OFF the PE: exp tiles are pre-summed
  on DVE (fp16 pair/quad tree) and reduced across partitions with one
  gpsimd partition_all_reduce per (head, query-tile); the diagonal
  128-block of each exp tile is masked by a 0/1 triangle multiply on DVE.
- score chunks are computed in PAIRS into 2-bank PSUM tiles so one Act
  exp instruction covers 1024 columns (halves Act instruction overhead --
  Act is the pacing engine during attention).
- projection chains whose outputs are consumed late (Q rows 0:512, used
  by the last-processed p=0 tile; K rows 1536:2048, used by p=3) are
  deferred out of phase 1 into phase 2, where they fill PE bubbles in the
  Act-paced attention stream.  Output-projection (Wo) chains fill the
  rest, paced per region.
"""

import os

import numpy as np

import concourse.bacc as bacc
import concourse.mybir as mybir
import concourse.tile as tile
from concourse import bass_isa
from concourse import bass_utils

_T = {
    "EPOOL": int(os.environ.get("K_EPOOL", "6")),
    "PEND": int(os.environ.get("K_PEND", "5")),
    "PSS": int(os.environ.get("K_PSS", "2")),
}

B, S, E, H = 2, 2048, 2048, 16
D = 128                    # head dim
HL = 4                     # heads per core
F = HL * D                 # local features = 512
EO = E // 128              # 16 contraction chunks
TT = 256                   # phase-1 token tile
IT = 512                   # phase-2 query tile
F32 = mybir.dt.float32
F16 = mybir.dt.float16
F8 = mybir.dt.float8e4
DR = mybir.MatmulPerfMode.DoubleRow
EXP = mybir.ActivationFunctionType.Exp
COPY = mybir.ActivationFunctionType.Copy
WS = 64.0                  # host-side weight prescale
SCALE = 1.0 / float(np.sqrt(D)) / (WS * WS)
YS = 1.0 / WS              # y de-scale: out1/2 unit scale, wo carries WS
NPF8 = mybir.dt.np(F8)

_CACHE = {}


def _build():
    nc = bacc.Bacc("TRN2", target_bir_lowering=False, debug=False)
    # x packed on host as [ei=128, tt, eo, TT] so each phase-1 tile loads as
    # one DMA with 4KB-contiguous per-partition payload (full bus efficiency)
    x1P = nc.dram_tensor("x1P", [128, S // TT, EO, TT], F8,
                         kind="ExternalInput").ap()
    x2P = nc.dram_tensor("x2P", [128, S // TT, EO, TT], F8,
                         kind="ExternalInput").ap()
    # all weights pre-packed on host into their SBUF layouts (contiguous
    # per-partition payloads -> full DMA bus efficiency); q/k are fc-major
    # so the first head-column can land in one small DMA at startup
    w_t = {
        n: nc.dram_tensor(n, [128, HL, EO, 128], F8, kind="ExternalInput").ap()
        for n in ("wq1", "wq2", "wk1", "wk2")
    }
    for n in ("wv1", "wv2"):
        w_t[n] = nc.dram_tensor(n, [128, EO, F], F8, kind="ExternalInput").ap()
    wo1_t = nc.dram_tensor("wo1", [128, HL, E], F8, kind="ExternalInput").ap()
    wo2_t = nc.dram_tensor("wo2", [128, HL, E], F8, kind="ExternalInput").ap()
    # 0/1 keep-mask for the k>q half of a diagonal scores block (DVE)
    tri_t = nc.dram_tensor("tri01", [128, 128], F16, kind="ExternalInput").ap()
    y = nc.dram_tensor("y", [S, E], mybir.dt.bfloat16, kind="ExternalOutput").ap()

    with tile.TileContext(nc) as tc:
        with tc.tile_pool(name="persist", bufs=1) as persist:
            qT = persist.tile([128, HL, S], F16, tag="qT")
            kT = persist.tile([128, HL, S], F16, tag="kT")
            vN = persist.tile([128, S // 128, F], F16, tag="vN")
            out1 = persist.tile([128, HL, S], F8, tag="out1")
            out2 = persist.tile([128, HL, S], F8, tag="out2")
            triT = persist.tile([128, 128], F16, tag="triT")

            # wq/wk + four x tiles persist into phase 2 for the deferred
            # projection chains (Q rows 0:512 -> consumed by p=0 last;
            # K rows 1536:2048 -> consumed by p=3)
            whold = tc.alloc_tile_pool(name="whold", bufs=1)
            xhold = tc.alloc_tile_pool(name="xhold", bufs=1)
            wres = {}
            for n in ("wq1", "wq2", "wk1", "wk2"):
                wres[n] = whold.tile([128, HL, EO, 128], F8, tag=n, name=n)
            xt = {}
            for tt in (0, 1, 6, 7):
                xt[tt] = (
                    xhold.tile([128, EO, TT], F8, tag=f"x1h{tt}",
                               name=f"x1h{tt}"),
                    xhold.tile([128, EO, TT], F8, tag=f"x2h{tt}",
                               name=f"x2h{tt}"),
                )

            # ---------- phase 1: q/k/v projections ------------------------
            with (
                tc.tile_pool(name="wres", bufs=1) as wpool,
                tc.tile_pool(name="xstream", bufs=4) as xpool,
                tc.tile_pool(name="ps_qk", bufs=5, space="PSUM") as ps_qk,
                tc.tile_pool(name="ps_v", bufs=2, space="PSUM") as ps_v,
            ):
                for n in ("wv1", "wv2"):
                    wres[n] = wpool.tile([128, EO, F], F8, tag=n, name=n)
                # startup order: the first K chain's inputs lead (wq is
                # deferred, so K/V data owns the bus); wv streams in
                # chunk-quarters interleaved with the later wk columns so
                # the tt=0 V chains aren't starved behind 1MB transfers
                nc.sync.dma_start(wres["wk1"][:, 0], w_t["wk1"][:, 0])
                nc.sync.dma_start(xt[0][0][:, 0:8], x1P[:, 0, 0:8])
                nc.sync.dma_start(xt[0][0][:, 8:], x1P[:, 0, 8:])
                nc.sync.dma_start(xt[0][1][:, 0:8], x2P[:, 0, 0:8])
                nc.sync.dma_start(xt[0][1][:, 8:], x2P[:, 0, 8:])
                nc.sync.dma_start(wres["wk2"][:, 0], w_t["wk2"][:, 0])
                nc.sync.dma_start(wres["wk1"][:, 1], w_t["wk1"][:, 1])
                nc.sync.dma_start(wres["wk2"][:, 1], w_t["wk2"][:, 1])
                nc.sync.dma_start(wres["wv1"][:, 0:4], w_t["wv1"][:, 0:4])
                nc.sync.dma_start(wres["wk1"][:, 2], w_t["wk1"][:, 2])
                nc.sync.dma_start(wres["wk2"][:, 2], w_t["wk2"][:, 2])
                nc.sync.dma_start(wres["wv1"][:, 4:8], w_t["wv1"][:, 4:8])
                nc.sync.dma_start(wres["wk1"][:, 3], w_t["wk1"][:, 3])
                nc.sync.dma_start(wres["wk2"][:, 3], w_t["wk2"][:, 3])
                nc.sync.dma_start(wres["wv1"][:, 8:], w_t["wv1"][:, 8:])
                nc.sync.dma_start(wres["wv2"][:, 0:8], w_t["wv2"][:, 0:8])
                nc.sync.dma_start(wres["wv2"][:, 8:], w_t["wv2"][:, 8:])
                nc.sync.dma_start(xt[1][0][:], x1P[:, 1])
                nc.sync.dma_start(xt[1][1][:], x2P[:, 1])
                nc.sync.dma_start(wres["wq1"][:], w_t["wq1"])
                nc.sync.dma_start(wres["wq2"][:], w_t["wq2"])
                deferred = []   # K chains first (earlier deadline), then Q
                for tt in range(S // TT):
                    t0 = tt * TT
                    if tt in xt:
                        x1, x2 = xt[tt]
                        if tt >= 6:
                            nc.sync.dma_start(x1[:], x1P[:, tt])
                            nc.sync.dma_start(x2[:], x2P[:, tt])
                    else:
                        x1 = xpool.tile([128, EO, TT], F8, tag="x1")
                        x2 = xpool.tile([128, EO, TT], F8, tag="x2")
                        nc.sync.dma_start(x1[:], x1P[:, tt])
                        nc.sync.dma_start(x2[:], x2P[:, tt])
                    if tt == 4:
                        nc.sync.dma_start(triT[:], tri_t)
                    for wn, dst in (("wq", qT), ("wk", kT)):
                        if wn == "wq" and tt < 2:
                            for fc in range(HL):
                                deferred.append(("wq", qT, fc, x1, x2, t0))
                            continue
                        if wn == "wk" and tt >= 6:
                            for fc in range(HL):
                                deferred.append(("wk", kT, fc, x1, x2, t0))
                            continue
                        w1, w2 = wres[wn + "1"], wres[wn + "2"]
                        for fc in range(HL):
                            ps = ps_qk.tile([128, TT], F32, tag="pqk")
                            terms = (
                                [(w1, x1, g) for g in range(0, EO, 2)]
                                + [(w1, x2, g) for g in range(0, EO, 2)]
                                + [(w2, x1, g) for g in range(0, EO, 2)]
                            )
                            for i, (w, x, g) in enumerate(terms):
                                nc.tensor.matmul(
                                    ps[:],
                                    w[:, fc, g:g + 2, :],
                                    x[:, g:g + 2, :],
                                    start=(i == 0),
                                    stop=(i == len(terms) - 1),
                                    perf_mode=DR,
                                )
                            nc.vector.tensor_copy(dst[:, fc, t0:t0 + TT], ps[:])
                    w1, w2 = wres["wv1"], wres["wv2"]
                    for tc2 in range(TT // 128):
                        tsl = slice(tc2 * 128, (tc2 + 1) * 128)
                        ps = ps_v.tile([128, F], F32, tag="pv")
                        terms = (
                            [(x1, w1, g) for g in range(0, EO, 2)]
                            + [(x2, w1, g) for g in range(0, EO, 2)]
                            + [(x1, w2, g) for g in range(0, EO, 2)]
                        )
                        for i, (x, w, g) in enumerate(terms):
                            nc.tensor.matmul(
                                ps[:],
                                x[:, g:g + 2, tsl],
                                w[:, g:g + 2, :],
                                start=(i == 0),
                                stop=(i == len(terms) - 1),
                                perf_mode=DR,
                            )
                        # v lands at unit scale (the wv prescale is undone
                        # here) so the normalized attention output fits
                        # fp8e4's range for the hi/lo split
                        nc.vector.tensor_scalar_mul(
                            vN[:, (t0 // 128) + tc2, :], ps[:], 1.0 / WS
                        )
                deferred.sort(key=lambda d: d[0] != "wk")

            # ---------- phase 2: attention per head ----------------------
            with tc.tile_pool(name="wo", bufs=1) as wo_pool:
                wo1_r = wo_pool.tile([128, HL, E], F8, tag="wo1")
                wo2_r = wo_pool.tile([128, HL, E], F8, tag="wo2")
                nc.sync.dma_start(wo1_r[:], wo1_t)
                nc.sync.dma_start(wo2_r[:], wo2_t)

                with (
                    tc.tile_pool(name="ph2", bufs=_T["EPOOL"]) as epool,
                    tc.tile_pool(name="ph2s", bufs=3) as spool,
                    tc.tile_pool(name="ph2t", bufs=2) as tpool,
                    tc.tile_pool(name="ph2b", bufs=2) as small,
                    tc.tile_pool(name="ph2f", bufs=3) as fpool,
                    tc.tile_pool(name="ps_s", bufs=_T["PSS"], space="PSUM") as ps_s,
                    tc.tile_pool(name="ps_o", bufs=2, space="PSUM") as ps_o,
                    tc.tile_pool(name="ps_yb", bufs=1, space="PSUM") as ps_yb,
                    tc.tile_pool(name="ps_q", bufs=1, space="PSUM") as ps_q,
                    tc.tile_pool(name="ystb", bufs=4) as ystb_pool,
                ):
                    ready_y = []

                    dq_state = {"cur": None, "idx": 0}

                    def emit_deferred_terms(nterms):
                        # deferred projection chains emitted in term-granular
                        # slices: fine-grained PE filler that matches the
                        # small per-pair Act deficit instead of overshooting
                        while nterms > 0:
                            if dq_state["cur"] is None:
                                if not deferred:
                                    return
                                wn, dst, fc, x1, x2, t0 = deferred.pop(0)
                                w1, w2 = wres[wn + "1"], wres[wn + "2"]
                                ps = ps_q.tile([128, TT], F32, tag="pq")
                                terms = (
                                    [(w1, x1, g) for g in range(0, EO, 2)]
                                    + [(w1, x2, g) for g in range(0, EO, 2)]
                                    + [(w2, x1, g) for g in range(0, EO, 2)]
                                )
                                dq_state["cur"] = (dst, fc, t0, ps, terms)
                                dq_state["idx"] = 0
                            dst, fc, t0, ps, terms = dq_state["cur"]
                            i0x = dq_state["idx"]
                            take = min(nterms, len(terms) - i0x)
                            for i in range(i0x, i0x + take):
                                w, x, g = terms[i]
                                nc.tensor.matmul(
                                    ps[:],
                                    w[:, fc, g:g + 2, :],
                                    x[:, g:g + 2, :],
                                    start=(i == 0),
                                    stop=(i == len(terms) - 1),
                                    perf_mode=DR,
                                )
                            dq_state["idx"] += take
                            nterms -= take
                            if dq_state["idx"] == len(terms):
                                nc.vector.tensor_copy(
                                    dst[:, fc, t0:t0 + TT], ps[:]
                                )
                                dq_state["cur"] = None

                    def emit_y_pair(split_dma=False, in_ph2=True,
                                    final=False):
                        # one (tcb, et-pair) group: two Wo psum chains into a
                        # single [128, 1024] bf16 store
                        tcb, ep = ready_y.pop(0)
                        tsl = slice(tcb * 128, (tcb + 1) * 128)
                        yb = ystb_pool.tile([128, 1024], mybir.dt.bfloat16,
                                            tag="yb")
                        for j in range(2):
                            esl = slice((2 * ep + j) * 512,
                                        (2 * ep + j + 1) * 512)
                            if j == 0 and in_ph2:
                                Ybt = ps_yb.tile([128, 512], F32, tag="Yb")
                                Ysl = lambda a, b: Ybt[:, a:b]
                            elif in_ph2:
                                # borrow half a scores-pair tile (same tag:
                                # no extra PSUM banks)
                                Ybt = ps_s.tile([128, 2, 512], F32, tag="S")
                                Ysl = lambda a, b: Ybt[:, 0, a:b]
                            elif j == 0:
                                # phase 3: the attention O banks are free --
                                # rotating them avoids waiting on the single
                                # ps_yb bank's Act drain
                                Ybt = ps_o.tile([128, IT], F32, tag="O")
                                Ysl = lambda a, b: Ybt[:, a:b]
                            else:
                                Ybt = ps_s.tile([128, 2, 512], F32, tag="S")
                                Ysl = lambda a, b: Ybt[:, 0, a:b]
                            Yb = Ysl(0, 512)
                            terms = []
                            for fp in range(HL // 2):
                                g = 2 * fp
                                terms += [(out1, wo1_r, g), (out2, wo1_r, g),
                                          (out1, wo2_r, g)]
                            for i, (o, w, g) in enumerate(terms):
                                nc.tensor.matmul(
                                    Yb,
                                    o[:, g:g + 2, tsl],
                                    w[:, g:g + 2, esl],
                                    start=(i == 0),
                                    stop=(i == len(terms) - 1),
                                    perf_mode=DR,
                                )
                            if final and j == 1:
                                # last store of the kernel: one Act scale,
                                # one [128,512] store -- every extra store
                                # costs a serial HWDGE slot (625ns), so no
                                # further splitting pays off
                                nc.scalar.activation(
                                    yb[:, 512:1024], Yb, COPY, scale=YS,
                                )
                                nc.sync.dma_start(y[tsl, esl],
                                                  yb[:, 512:1024])
                                continue
                            if j == 0:
                                # Act: keeps the single ps_yb bank's drain
                                # off DVE's in-order queue
                                nc.scalar.activation(
                                    yb[:, 0:512], Yb, COPY, scale=YS,
                                )
                            else:
                                # DVE (gpsimd cannot read PSUM)
                                nc.vector.tensor_scalar_mul(
                                    yb[:, 512:1024], Yb, YS
                                )
                            if split_dma or final:
                                nc.sync.dma_start(y[tsl, esl],
                                                  yb[:, j * 512:(j + 1) * 512])
                        if not (split_dma or final):
                            nc.sync.dma_start(
                                y[tsl, ep * 1024:(ep + 1) * 1024], yb[:]
                            )

                    def emit_y_quad():
                        # phase 3 only: two adjacent (tcb, ep=0/1) groups,
                        # four Wo chains, ONE [128, 2048] store -- halves the
                        # per-store issue overhead (SP config + HWDGE are a
                        # shared serial resource that otherwise can't keep
                        # pace with the chain cadence)
                        tcb, _ = ready_y.pop(0)
                        ready_y.pop(0)
                        tsl = slice(tcb * 128, (tcb + 1) * 128)
                        yb2 = ystb_pool.tile([128, 2048], mybir.dt.bfloat16,
                                             tag="yb2")
                        for half in range(4):
                            esl = slice(half * 512, (half + 1) * 512)
                            if half % 2 == 0:
                                Ybt = ps_o.tile([128, IT], F32, tag="O")
                                Yb = Ybt[:]
                            else:
                                Ybt = ps_s.tile([128, 2, 512], F32, tag="S")
                                Yb = Ybt[:, 0, :]
                            terms = []
                            for fp in range(HL // 2):
                                g = 2 * fp
                                terms += [(out1, wo1_r, g), (out2, wo1_r, g),
                                          (out1, wo2_r, g)]
                            for i, (o, w, g) in enumerate(terms):
                                nc.tensor.matmul(
                                    Yb,
                                    o[:, g:g + 2, tsl],
                                    w[:, g:g + 2, esl],
                                    start=(i == 0),
                                    stop=(i == len(terms) - 1),
                                    perf_mode=DR,
                                )
                            if half % 2 == 0:
                                nc.scalar.activation(
                                    yb2[:, esl], Yb, COPY, scale=YS,
                                )
                            else:
                                nc.vector.tensor_scalar_mul(
                                    yb2[:, esl], Yb, YS
                                )
                        nc.sync.dma_start(y[tsl, :], yb2[:])

                    deferred_split = []

                    def flush_split():
                        while deferred_split:
                            Ocp_d, h_d, i0_d = deferred_split.pop(0)
                            # hi/lo split on Pool: keeps the per-head
                            # normalize chain off DVE's in-order queue
                            nc.gpsimd.tensor_copy(
                                out1[:, h_d, i0_d:i0_d + IT], Ocp_d[:]
                            )
                            nc.gpsimd.tensor_sub(
                                out2[:, h_d, i0_d:i0_d + IT], Ocp_d[:],
                                out1[:, h_d, i0_d:i0_d + IT],
                            )
                            if h_d == HL - 1:
                                # the head-group's outputs are now all
                                # written -- its y pairs may be emitted
                                pd = i0_d // IT
                                for tcb_r in range(4 * pd, 4 * pd + 4):
                                    for ep_r in range(E // 1024):
                                        ready_y.append((tcb_r, ep_r))

                    # p0 (shortest, latency-bound) runs last, when y-pair
                    # chains exist to fill PE while Act/Pool/DVE drain
                    p_order = (1, 2, 3, 0)
                    for pi, p in enumerate(p_order):
                        i0 = p * IT
                        for h in range(HL):
                            h0 = h * 128
                            njc = (i0 + IT) // 128
                            npair = njc // 2
                            ndiag0 = i0 // 128   # first diag chunk index
                            O = ps_o.tile([128, IT], F32, tag="O")
                            # fp16 running rowsum accumulator; reduced
                            # across partitions at the end by one gpsimd
                            # all-reduce
                            T = tpool.tile([128, IT], F16, tag="T")

                            pending = []
                            st = {"t": False, "es": None}

                            def fill_slot(u=None, at_flush=False):
                                # PE filler: term-granular deferred chains
                                # in the Act-paced early regions, Wo chains
                                # later; p=0 fills only after its exps (so
                                # Act's in-order queue stays on exp) and
                                # keeps 2 pairs back to bridge into phase 3
                                if p == 0:
                                    if len(ready_y) > 3:
                                        emit_y_pair()
                                elif p == 3:
                                    if (not at_flush and u in (1, npair - 1)
                                            and ready_y):
                                        emit_y_pair()
                                    elif at_flush and len(ready_y) > 6:
                                        emit_y_pair()
                                    elif not at_flush and u == 5:
                                        emit_deferred_terms(24)
                                elif p == 2:
                                    if at_flush and len(ready_y) > 6:
                                        emit_y_pair()
                                    elif not at_flush and u in (1, 3):
                                        emit_deferred_terms(24)
                                else:
                                    if not at_flush and u in (1, 3):
                                        emit_deferred_terms(24)

                            def emit_av(jc, Et2, i, off):
                                nc.tensor.matmul(
                                    O[:, off:],
                                    vN[:, jc, h0:h0 + 128],
                                    Et2[:, i, off:],
                                    start=(jc == 0),
                                    stop=(jc == njc - 1),
                                )

                            for u in range(npair):
                                jc0 = 2 * u
                                fill_slot(u=u)
                                # --- scores pair ---------------------------
                                ps2 = ps_s.tile([128, 2, 512], F32, tag="S")
                                Et2 = epool.tile([128, 2, 512], F16, tag="E2")
                                for i in range(2):
                                    jc = jc0 + i
                                    q_off = jc - ndiag0
                                    off = 0 if q_off < 0 else 128 * q_off
                                    nc.tensor.matmul(
                                        ps2[:, i, off:],
                                        kT[:, h, jc * 128:(jc + 1) * 128],
                                        qT[:, h, i0 + off:i0 + IT],
                                        start=True,
                                        stop=True,
                                    )
                                if jc0 + 1 < ndiag0:
                                    # non-diag pair: one exp over both banks
                                    nc.scalar.activation(
                                        Et2[:, :, :], ps2[:, :, :], EXP,
                                        scale=SCALE,
                                    )
                                    EtS = spool.tile([128, IT], F16,
                                                     tag="EtS")
                                    nc.vector.tensor_add(
                                        EtS[:], Et2[:, 0, :], Et2[:, 1, :]
                                    )
                                    if u % 2 == 1:
                                        if not st["t"]:
                                            nc.vector.tensor_add(
                                                T[:], st["es"][:], EtS[:]
                                            )
                                            st["t"] = True
                                        else:
                                            EtQ = spool.tile(
                                                [128, IT], F16, tag="EtQ")
                                            nc.vector.tensor_add(
                                                EtQ[:], st["es"][:], EtS[:]
                                            )
                                            nc.vector.tensor_add(
                                                T[:], T[:], EtQ[:]
                                            )
                                    st["es"] = EtS
                                    pending.append((jc0, Et2, 0, 0))
                                    pending.append((jc0 + 1, Et2, 1, 0))
                                else:
                                    # diag pair: ragged exps + mask + T
                                    for i in range(2):
                                        jc = jc0 + i
                                        off = 128 * (jc - ndiag0)
                                        nc.scalar.activation(
                                            Et2[:, i, off:],
                                            ps2[:, i, off:], EXP,
                                            scale=SCALE,
                                        )
                                        nc.vector.tensor_mul(
                                            Et2[:, i, off:off + 128],
                                            Et2[:, i, off:off + 128],
                                            triT[:],
                                        )
                                        if not st["t"]:
                                            nc.vector.tensor_copy(
                                                T[:], Et2[:, i, :]
                                            )
                                            st["t"] = True
                                        else:
                                            nc.vector.tensor_add(
                                                T[:, off:], T[:, off:],
                                                Et2[:, i, off:],
                                            )
                                        pending.append((jc, Et2, i, off))
                                while len(pending) > _T["PEND"]:
                                    emit_av(*pending.pop(0))
                            fill_slot(at_flush=True)
                            for item in pending:
                                emit_av(*item)
                            # rowsum across k (partitions) in one gpsimd op,
                            # broadcast to all partitions; then 1/sum on DVE
                            Rb = small.tile([128, IT], F32, tag="Rb")
                            nc.gpsimd.partition_all_reduce(
                                Rb[:], T[:], channels=128,
                                reduce_op=bass_isa.ReduceOp.add,
                            )
                            rec = small.tile([128, IT], F32, tag="rec")
                            nc.vector.reciprocal(rec[:], Rb[:])
                            Ocp = fpool.tile([128, IT], F32, tag="Ocp")
                            nc.vector.tensor_mul(Ocp[:], O[:], rec[:])
                            # defer the fp8 hi/lo split of this iteration's
                            # output until the next iteration, so Pool's
                            # all-reduce is never queued behind a hi-copy;
                            # the last head of each row-block flushes
                            # immediately so its y-pairs unlock before the
                            # next region's filler slots
                            flush_split()
                            deferred_split.append((Ocp, h, i0))
                            if h == HL - 1:
                                flush_split()

                    # ---- phase 3: remaining output-projection groups ------
                    # batch stores two-pairs-at-a-time; the last two pairs
                    # split/quarter their stores so the post-PE drain is
                    # short
                    while ready_y:
                        if (len(ready_y) > 4
                                and ready_y[0][0] == ready_y[1][0]
                                and ready_y[0][1] == 0
                                and ready_y[1][1] == 1):
                            emit_y_quad()
                        else:
                            emit_y_pair(split_dma=False, in_ph2=False,
                                        final=(len(ready_y) == 1))
            xhold.release()
            whold.release()
    nc.compile()
    return nc


def _get_nc():
    if "nc" not in _CACHE:
        _CACHE["nc"] = _build()
    return _CACHE["nc"]


def _split8(a):
    hi = a.astype(NPF8)
    lo = (a - hi.astype(np.float32)).astype(NPF8)
    return hi, lo


def make_in_maps(x, Wq, Wk, Wv, Wo):
    x = np.asarray(x, np.float32)
    Wq = np.asarray(Wq, np.float32)
    Wk = np.asarray(Wk, np.float32)
    Wv = np.asarray(Wv, np.float32)
    Wo = np.asarray(Wo, np.float32)

    # keep-mask in [k_local (partition), q_local (col)] layout: keep k<=q
    tri01 = np.ascontiguousarray(np.triu(np.ones((128, 128), np.float16)))

    def _pack_x(xT8):
        # [E, S] -> [ei=128, tt, eo, TT] (phase-1 tile DMA order)
        return np.ascontiguousarray(
            xT8.reshape(EO, 128, S // TT, TT).transpose(1, 2, 0, 3)
        )

    xs = []
    for b in range(B):
        hi, lo = _split8(np.ascontiguousarray(x[b].T))
        xs.append((_pack_x(hi), _pack_x(lo)))
    def _pack_qk(w8):  # [E, F] -> [ei=128, fc, eo, 128]
        return np.ascontiguousarray(
            w8.reshape(EO, 128, HL, 128).transpose(1, 2, 0, 3)
        )

    def _pack_v(w8):  # [E, F] -> [ei=128, eo, F]
        return np.ascontiguousarray(
            w8.reshape(EO, 128, F).transpose(1, 0, 2)
        )

    def _pack_o(w8):  # [F, E] -> [fi=128, fc, E]
        return np.ascontiguousarray(
            w8.reshape(HL, 128, E).transpose(1, 0, 2)
        )

    in_maps = []
    for c in range(8):
        b, g = c // 4, c % 4
        fsl = slice(F * g, F * (g + 1))
        m = {"x1P": xs[b][0], "x2P": xs[b][1], "tri01": tri01}
        for n, W, pk in (("wq", Wq, _pack_qk), ("wk", Wk, _pack_qk),
                         ("wv", Wv, _pack_v)):
            hi, lo = _split8(np.ascontiguousarray(W[fsl, :].T) * WS)
            m[n + "1"], m[n + "2"] = pk(hi), pk(lo)
        hi, lo = _split8(np.ascontiguousarray(Wo[:, fsl].T) * WS)
        m["wo1"], m["wo2"] = _pack_o(hi), _pack_o(lo)
        in_maps.append(m)
    return in_maps


def combine_outputs(results):
    out = np.empty((B, S, E), np.float32)
    for b in range(B):
        acc = results[4 * b]["y"].astype(np.float32).copy()
        for g in range(1, 4):
            acc += results[4 * b + g]["y"]
        out[b] = acc
    return out


def kernel(x, Wq, Wk, Wv, Wo):
    import time as _time

    nc = _get_nc()
    in_maps = make_in_maps(x, Wq, Wk, Wv, Wo)
    last_exc = None
    for attempt in range(3):
        if attempt:
            # transient device wedge (e.g. NRT_EXEC_UNIT_UNRECOVERABLE) --
            # wait for recovery before retrying
            _time.sleep(30 * attempt)
        try:
            res = bass_utils.run_bass_kernel_spmd(
                nc, in_maps, core_ids=list(range(8))
            )
            return combine_outputs(res.results)
        except Exception as exc:
            last_exc = exc
    raise last_exc


# revision 66
# speedup vs baseline: 1.0474x; 1.0066x over previous
"""Multi-head attention (B=2, S=2048, E=2048, H=16, causal) on 8 TRN2 NeuronCores.

Sharding: 8 cores = 2 batch shards x 4 head-group shards (4 heads / 512
features each).  Each core runs the full attention stack for its (batch,
head-group) and produces a partial [S, E] output through its row-block of
Wo; the host sums the 4 partials per batch.

Projections (QKV, Wo) run as 3-term fp8e4m3 hi/lo split matmuls in
DoubleRow perf mode (A@B ~ A1B1 + A2B1 + A1B2, each term contracting
2x128 rows per pass at 0.5 cyc/row).  The hi/lo splits of x and the
weights are prepared on the host; weights are pre-scaled by 64 so their
values sit in fp8's normal range (the scale is undone on the way out).
Attention (scores, attn@v) runs in fp16 at full PE rate.

Scheduling structure:
- softmax rowsum and causal mask run OFF the PE: exp tiles are pre-summed
  on DVE (fp16 pair/quad tree) and reduced across partitions with one
  gpsimd partition_all_reduce per (head, query-tile); the diagonal
  128-block of each exp tile is masked by a 0/1 triangle multiply on DVE.
- score chunks are computed in PAIRS into 2-bank PSUM tiles so one Act
  exp instruction covers 1024 columns (halves Act instruction overhead --
  Act is the pacing engine during attention).
- projection chains whose outputs are consumed late (Q rows 0:512, used
  by the last-processed p=0 tile; K rows 1536:2048, used by p=3) are
  deferred out of phase 1 into phase 2, where they fill PE bubbles in the
  Act-paced attention stream.  Output-projection (Wo) chains fill the
  rest, paced per region.
"""

import os

import numpy as np

import concourse.bacc as bacc
import concourse.mybir as mybir
import concourse.tile as tile
from concourse import bass_isa
from concourse import bass_utils

_T = {
    "EPOOL": int(os.environ.get("K_EPOOL", "6")),
    "PEND": int(os.environ.get("K_PEND", "5")),
    "PSS": int(os.environ.get("K_PSS", "3")),
}

B, S, E, H = 2, 2048, 2048, 16
D = 128                    # head dim
HL = 4                     # heads per core
F = HL * D                 # local features = 512
EO = E // 128              # 16 contraction chunks
TT = 256                   # phase-1 token tile
IT = 512                   # phase-2 query tile
F32 = mybir.dt.float32
F16 = mybir.dt.float16
F8 = mybir.dt.float8e4
DR = mybir.MatmulPerfMode.DoubleRow
EXP = mybir.ActivationFunctionType.Exp
COPY = mybir.ActivationFunctionType.Copy
WS = 64.0                  # host-side weight prescale
SCALE = 1.0 / float(np.sqrt(D)) / (WS * WS)
YS = 1.0 / WS              # y de-scale: out1/2 unit scale, wo carries WS
NPF8 = mybir.dt.np(F8)

_CACHE = {}


def _build():
    nc = bacc.Bacc("TRN2", target_bir_lowering=False, debug=False)
    # x packed on host as [ei=128, tt, eo, TT] so each phase-1 tile loads as
    # one DMA with 4KB-contiguous per-partition payload (full bus efficiency)
    x1P = nc.dram_tensor("x1P", [128, S // TT, EO, TT], F8,
                         kind="ExternalInput").ap()
    x2P = nc.dram_tensor("x2P", [128, S // TT, EO, TT], F8,
                         kind="ExternalInput").ap()
    # all weights pre-packed on host into their SBUF layouts (contiguous
    # per-partition payloads -> full DMA bus efficiency); q/k are fc-major
    # so the first head-column can land in one small DMA at startup
    w_t = {
        n: nc.dram_tensor(n, [128, HL, EO, 128], F8, kind="ExternalInput").ap()
        for n in ("wq1", "wq2", "wk1", "wk2")
    }
    for n in ("wv1", "wv2"):
        w_t[n] = nc.dram_tensor(n, [128, EO, F], F8, kind="ExternalInput").ap()
    wo1_t = nc.dram_tensor("wo1", [128, HL, E], F8, kind="ExternalInput").ap()
    wo2_t = nc.dram_tensor("wo2", [128, HL, E], F8, kind="ExternalInput").ap()
    # 0/1 keep-mask for the k>q half of a diagonal scores block (DVE)
    tri_t = nc.dram_tensor("tri01", [128, 128], F16, kind="ExternalInput").ap()
    y = nc.dram_tensor("y", [S, E], mybir.dt.bfloat16, kind="ExternalOutput").ap()

    with tile.TileContext(nc) as tc:
        with tc.tile_pool(name="persist", bufs=1) as persist:
            qT = persist.tile([128, HL, S], F16, tag="qT")
            kT = persist.tile([128, HL, S], F16, tag="kT")
            vN = persist.tile([128, S // 128, F], F16, tag="vN")
            out1 = persist.tile([128, HL, S], F8, tag="out1")
            out2 = persist.tile([128, HL, S], F8, tag="out2")
            triT = persist.tile([128, 128], F16, tag="triT")

            # wq/wk + four x tiles persist into phase 2 for the deferred
            # projection chains (Q rows 0:512 -> consumed by p=0 last;
            # K rows 1536:2048 -> consumed by p=3)
            whold = tc.alloc_tile_pool(name="whold", bufs=1)
            xhold = tc.alloc_tile_pool(name="xhold", bufs=1)
            wres = {}
            for n in ("wq1", "wq2", "wk1", "wk2"):
                wres[n] = whold.tile([128, HL, EO, 128], F8, tag=n, name=n)
            xt = {}
            for tt in (0, 1, 6, 7):
                xt[tt] = (
                    xhold.tile([128, EO, TT], F8, tag=f"x1h{tt}",
                               name=f"x1h{tt}"),
                    xhold.tile([128, EO, TT], F8, tag=f"x2h{tt}",
                               name=f"x2h{tt}"),
                )

            # ---------- phase 1: q/k/v projections ------------------------
            with (
                tc.tile_pool(name="wres", bufs=1) as wpool,
                tc.tile_pool(name="xstream", bufs=4) as xpool,
                tc.tile_pool(name="ps_qk", bufs=5, space="PSUM") as ps_qk,
                tc.tile_pool(name="ps_v", bufs=2, space="PSUM") as ps_v,
            ):
                for n in ("wv1", "wv2"):
                    wres[n] = wpool.tile([128, EO, F], F8, tag=n, name=n)
                # startup order: the first K chain's inputs lead (wq is
                # deferred, so K/V data owns the bus); wv streams in
                # chunk-quarters interleaved with the later wk columns so
                # the tt=0 V chains aren't starved behind 1MB transfers
                nc.sync.dma_start(wres["wk1"][:, 0], w_t["wk1"][:, 0])
                nc.sync.dma_start(xt[0][0][:, 0:8], x1P[:, 0, 0:8])
                nc.sync.dma_start(xt[0][0][:, 8:], x1P[:, 0, 8:])
                nc.sync.dma_start(xt[0][1][:, 0:8], x2P[:, 0, 0:8])
                nc.sync.dma_start(xt[0][1][:, 8:], x2P[:, 0, 8:])
                nc.sync.dma_start(wres["wk2"][:, 0], w_t["wk2"][:, 0])
                nc.sync.dma_start(wres["wk1"][:, 1], w_t["wk1"][:, 1])
                nc.sync.dma_start(wres["wk2"][:, 1], w_t["wk2"][:, 1])
                nc.sync.dma_start(wres["wv1"][:, 0:4], w_t["wv1"][:, 0:4])
                nc.sync.dma_start(wres["wk1"][:, 2], w_t["wk1"][:, 2])
                nc.sync.dma_start(wres["wk2"][:, 2], w_t["wk2"][:, 2])
                nc.sync.dma_start(wres["wv1"][:, 4:8], w_t["wv1"][:, 4:8])
                nc.sync.dma_start(wres["wk1"][:, 3], w_t["wk1"][:, 3])
                nc.sync.dma_start(wres["wk2"][:, 3], w_t["wk2"][:, 3])
                nc.sync.dma_start(wres["wv1"][:, 8:], w_t["wv1"][:, 8:])
                nc.sync.dma_start(wres["wv2"][:, 0:8], w_t["wv2"][:, 0:8])
                nc.sync.dma_start(wres["wv2"][:, 8:], w_t["wv2"][:, 8:])
                nc.sync.dma_start(xt[1][0][:], x1P[:, 1])
                nc.sync.dma_start(xt[1][1][:], x2P[:, 1])
                nc.sync.dma_start(wres["wq1"][:], w_t["wq1"])
                nc.sync.dma_start(wres["wq2"][:], w_t["wq2"])
                deferred = []   # K chains first (earlier deadline), then Q
                for tt in range(S // TT):
                    t0 = tt * TT
                    if tt in xt:
                        x1, x2 = xt[tt]
                        if tt >= 6:
                            nc.sync.dma_start(x1[:], x1P[:, tt])
                            nc.sync.dma_start(x2[:], x2P[:, tt])
                    else:
                        x1 = xpool.tile([128, EO, TT], F8, tag="x1")
                        x2 = xpool.tile([128, EO, TT], F8, tag="x2")
                        nc.sync.dma_start(x1[:], x1P[:, tt])
                        nc.sync.dma_start(x2[:], x2P[:, tt])
                    if tt == 4:
                        nc.sync.dma_start(triT[:], tri_t)
                    for wn, dst in (("wq", qT), ("wk", kT)):
                        if wn == "wq" and tt < 2:
                            for fc in range(HL):
                                deferred.append(("wq", qT, fc, x1, x2, t0))
                            continue
                        if wn == "wk" and tt >= 6:
                            for fc in range(HL):
                                deferred.append(("wk", kT, fc, x1, x2, t0))
                            continue
                        w1, w2 = wres[wn + "1"], wres[wn + "2"]
                        for fc in range(HL):
                            ps = ps_qk.tile([128, TT], F32, tag="pqk")
                            terms = (
                                [(w1, x1, g) for g in range(0, EO, 2)]
                                + [(w1, x2, g) for g in range(0, EO, 2)]
                                + [(w2, x1, g) for g in range(0, EO, 2)]
                            )
                            for i, (w, x, g) in enumerate(terms):
                                nc.tensor.matmul(
                                    ps[:],
                                    w[:, fc, g:g + 2, :],
                                    x[:, g:g + 2, :],
                                    start=(i == 0),
                                    stop=(i == len(terms) - 1),
                                    perf_mode=DR,
                                )
                            nc.vector.tensor_copy(dst[:, fc, t0:t0 + TT], ps[:])
                    w1, w2 = wres["wv1"], wres["wv2"]
                    for tc2 in range(TT // 128):
                        tsl = slice(tc2 * 128, (tc2 + 1) * 128)
                        ps = ps_v.tile([128, F], F32, tag="pv")
                        terms = (
                            [(x1, w1, g) for g in range(0, EO, 2)]
                            + [(x2, w1, g) for g in range(0, EO, 2)]
                            + [(x1, w2, g) for g in range(0, EO, 2)]
                        )
                        for i, (x, w, g) in enumerate(terms):
                            nc.tensor.matmul(
                                ps[:],
                                x[:, g:g + 2, tsl],
                                w[:, g:g + 2, :],
                                start=(i == 0),
                                stop=(i == len(terms) - 1),
                                perf_mode=DR,
                            )
                        # v lands at unit scale (the wv prescale is undone
                        # here) so the normalized attention output fits
                        # fp8e4's range for the hi/lo split
                        nc.vector.tensor_scalar_mul(
                            vN[:, (t0 // 128) + tc2, :], ps[:], 1.0 / WS
                        )
                deferred.sort(key=lambda d: d[0] != "wk")

            # ---------- phase 2: attention per head ----------------------
            with tc.tile_pool(name="wo", bufs=1) as wo_pool:
                wo1_r = wo_pool.tile([128, HL, E], F8, tag="wo1")
                wo2_r = wo_pool.tile([128, HL, E], F8, tag="wo2")
                nc.sync.dma_start(wo1_r[:], wo1_t)
                nc.sync.dma_start(wo2_r[:], wo2_t)

                with (
                    tc.tile_pool(name="ph2", bufs=_T["EPOOL"]) as epool,
                    tc.tile_pool(name="ph2s", bufs=3) as spool,
                    tc.tile_pool(name="ph2t", bufs=2) as tpool,
                    tc.tile_pool(name="ph2b", bufs=2) as small,
                    tc.tile_pool(name="ph2f", bufs=3) as fpool,
                    tc.tile_pool(name="ps_s", bufs=_T["PSS"], space="PSUM") as ps_s,
                    tc.tile_pool(name="ps_o", bufs=2, space="PSUM") as ps_o,
                    tc.tile_pool(name="ystb", bufs=4) as ystb_pool,
                ):
                    ready_y = []

                    dq_state = {"cur": None, "idx": 0}

                    def emit_deferred_terms(nterms):
                        # deferred projection chains emitted in term-granular
                        # slices: fine-grained PE filler that matches the
                        # small per-pair Act deficit instead of overshooting
                        while nterms > 0:
                            if dq_state["cur"] is None:
                                if not deferred:
                                    return
                                wn, dst, fc, x1, x2, t0 = deferred.pop(0)
                                w1, w2 = wres[wn + "1"], wres[wn + "2"]
                                pst = ps_s.tile([128, 2, 512], F32,
                                                tag="S")
                                ps = pst[:, 0, 0:TT]
                                terms = (
                                    [(w1, x1, g) for g in range(0, EO, 2)]
                                    + [(w1, x2, g) for g in range(0, EO, 2)]
                                    + [(w2, x1, g) for g in range(0, EO, 2)]
                                )
                                dq_state["cur"] = (dst, fc, t0, ps, terms)
                                dq_state["idx"] = 0
                            dst, fc, t0, ps, terms = dq_state["cur"]
                            i0x = dq_state["idx"]
                            take = min(nterms, len(terms) - i0x)
                            for i in range(i0x, i0x + take):
                                w, x, g = terms[i]
                                nc.tensor.matmul(
                                    ps,
                                    w[:, fc, g:g + 2, :],
                                    x[:, g:g + 2, :],
                                    start=(i == 0),
                                    stop=(i == len(terms) - 1),
                                    perf_mode=DR,
                                )
                            dq_state["idx"] += take
                            nterms -= take
                            if dq_state["idx"] == len(terms):
                                nc.vector.tensor_copy(
                                    dst[:, fc, t0:t0 + TT], ps
                                )
                                dq_state["cur"] = None

                    def emit_y_pair(split_dma=False, in_ph2=True,
                                    final=False):
                        # one (tcb, et-pair) group: two Wo psum chains into a
                        # single [128, 1024] bf16 store
                        tcb, ep = ready_y.pop(0)
                        tsl = slice(tcb * 128, (tcb + 1) * 128)
                        yb = ystb_pool.tile([128, 1024], mybir.dt.bfloat16,
                                            tag="yb")
                        if in_ph2:
                            # both chains share one scores-pair tile (a
                            # half each): no dedicated y bank, so scores
                            # keep a 3-buffer rotation
                            Ypair = ps_s.tile([128, 2, 512], F32, tag="S")
                        for j in range(2):
                            esl = slice((2 * ep + j) * 512,
                                        (2 * ep + j + 1) * 512)
                            if in_ph2:
                                Ysl = (lambda jj: lambda a, b:
                                       Ypair[:, jj, a:b])(j)
                            else:
                                # phase 3: the attention O banks are free
                                Ybt = ps_o.tile([128, IT], F32, tag="O")
                                Ysl = (lambda t: lambda a, b: t[:, a:b])(Ybt)
                            Yb = Ysl(0, 512)
                            terms = []
                            for fp in range(HL // 2):
                                g = 2 * fp
                                terms += [(out1, wo1_r, g), (out2, wo1_r, g),
                                          (out1, wo2_r, g)]
                            for i, (o, w, g) in enumerate(terms):
                                nc.tensor.matmul(
                                    Yb,
                                    o[:, g:g + 2, tsl],
                                    w[:, g:g + 2, esl],
                                    start=(i == 0),
                                    stop=(i == len(terms) - 1),
                                    perf_mode=DR,
                                )
                            if final and j == 1:
                                # last store of the kernel: one Act scale,
                                # one [128,512] store -- every extra store
                                # costs a serial HWDGE slot (625ns), so no
                                # further splitting pays off
                                nc.scalar.activation(
                                    yb[:, 512:1024], Yb, COPY, scale=YS,
                                )
                                nc.sync.dma_start(y[tsl, esl],
                                                  yb[:, 512:1024])
                                continue
                            if j == 0:
                                # Act: keeps the single ps_yb bank's drain
                                # off DVE's in-order queue
                                nc.scalar.activation(
                                    yb[:, 0:512], Yb, COPY, scale=YS,
                                )
                            else:
                                # DVE (gpsimd cannot read PSUM)
                                nc.vector.tensor_scalar_mul(
                                    yb[:, 512:1024], Yb, YS
                                )
                            if split_dma or final:
                                nc.sync.dma_start(y[tsl, esl],
                                                  yb[:, j * 512:(j + 1) * 512])
                        if not (split_dma or final):
                            nc.sync.dma_start(
                                y[tsl, ep * 1024:(ep + 1) * 1024], yb[:]
                            )

                    def emit_y_quad():
                        # phase 3 only: two adjacent (tcb, ep=0/1) groups,
                        # four Wo chains, ONE [128, 2048] store -- halves the
                        # per-store issue overhead (SP config + HWDGE are a
                        # shared serial resource that otherwise can't keep
                        # pace with the chain cadence)
                        tcb, _ = ready_y.pop(0)
                        ready_y.pop(0)
                        tsl = slice(tcb * 128, (tcb + 1) * 128)
                        yb2 = ystb_pool.tile([128, 2048], mybir.dt.bfloat16,
                                             tag="yb2")
                        for half in range(4):
                            esl = slice(half * 512, (half + 1) * 512)
                            if half % 2 == 0:
                                Ybt = ps_o.tile([128, IT], F32, tag="O")
                                Yb = Ybt[:]
                            else:
                                Ybt = ps_s.tile([128, 2, 512], F32, tag="S")
                                Yb = Ybt[:, 0, :]
                            terms = []
                            for fp in range(HL // 2):
                                g = 2 * fp
                                terms += [(out1, wo1_r, g), (out2, wo1_r, g),
                                          (out1, wo2_r, g)]
                            for i, (o, w, g) in enumerate(terms):
                                nc.tensor.matmul(
                                    Yb,
                                    o[:, g:g + 2, tsl],
                                    w[:, g:g + 2, esl],
                                    start=(i == 0),
                                    stop=(i == len(terms) - 1),
                                    perf_mode=DR,
                                )
                            if half % 2 == 0:
                                nc.scalar.activation(
                                    yb2[:, esl], Yb, COPY, scale=YS,
                                )
                            else:
                                nc.vector.tensor_scalar_mul(
                                    yb2[:, esl], Yb, YS
                                )
                        nc.sync.dma_start(y[tsl, :], yb2[:])

                    deferred_split = []

                    def flush_split(on_dve=False):
                        while deferred_split:
                            Ocp_d, h_d, i0_d = deferred_split.pop(0)
                            # hi/lo split on Pool: keeps the per-head
                            # normalize chain off DVE's in-order queue.
                            # The release-critical last head of a row-block
                            # splits on DVE instead (shorter chain -> its
                            # y-pairs unlock sooner)
                            eng = nc.vector if on_dve else nc.gpsimd
                            eng.tensor_copy(
                                out1[:, h_d, i0_d:i0_d + IT], Ocp_d[:]
                            )
                            eng.tensor_sub(
                                out2[:, h_d, i0_d:i0_d + IT], Ocp_d[:],
                                out1[:, h_d, i0_d:i0_d + IT],
                            )
                            if h_d == HL - 1:
                                # the head-group's outputs are now all
                                # written -- its y pairs may be emitted
                                pd = i0_d // IT
                                for tcb_r in range(4 * pd, 4 * pd + 4):
                                    for ep_r in range(E // 1024):
                                        ready_y.append((tcb_r, ep_r))

                    # p0 (shortest, latency-bound) runs last, when y-pair
                    # chains exist to fill PE while Act/Pool/DVE drain
                    p_order = (1, 2, 0, 3)
                    for pi, p in enumerate(p_order):
                        i0 = p * IT
                        for h in range(HL):
                            h0 = h * 128
                            njc = (i0 + IT) // 128
                            npair = njc // 2
                            ndiag0 = i0 // 128   # first diag chunk index
                            O = ps_o.tile([128, IT], F32, tag="O")
                            # fp16 running rowsum accumulator; reduced
                            # across partitions at the end by one gpsimd
                            # all-reduce
                            T = tpool.tile([128, IT], F16, tag="T")

                            pending = []
                            st = {"t": False, "es": None}

                            def fill_slot(u=None, at_flush=False):
                                # PE filler: term-granular deferred chains
                                # in the Act-paced early regions, Wo chains
                                # later; p=0 fills only after its exps (so
                                # Act's in-order queue stays on exp) and
                                # keeps 2 pairs back to bridge into phase 3
                                if p == 0:
                                    if len(ready_y) > 3:
                                        emit_y_pair()
                                elif p == 3:
                                    if (not at_flush and u in (1, npair - 1)
                                            and len(ready_y) > 3):
                                        emit_y_pair()
                                    elif at_flush and len(ready_y) > 6:
                                        emit_y_pair()
                                    elif not at_flush and u == 5:
                                        emit_deferred_terms(24)
                                elif p == 2:
                                    if at_flush and len(ready_y) > 6:
                                        emit_y_pair()
                                    elif not at_flush and u in (1, 3):
                                        emit_deferred_terms(24)
                                else:
                                    if not at_flush and u in (1, 3):
                                        emit_deferred_terms(24)

                            def emit_av(jc, Et2, i, off):
                                nc.tensor.matmul(
                                    O[:, off:],
                                    vN[:, jc, h0:h0 + 128],
                                    Et2[:, i, off:],
                                    start=(jc == 0),
                                    stop=(jc == njc - 1),
                                )

                            for u in range(npair):
                                jc0 = 2 * u
                                fill_slot(u=u)
                                # --- scores pair ---------------------------
                                ps2 = ps_s.tile([128, 2, 512], F32, tag="S")
                                Et2 = epool.tile([128, 2, 512], F16, tag="E2")
                                for i in range(2):
                                    jc = jc0 + i
                                    q_off = jc - ndiag0
                                    off = 0 if q_off < 0 else 128 * q_off
                                    nc.tensor.matmul(
                                        ps2[:, i, off:],
                                        kT[:, h, jc * 128:(jc + 1) * 128],
                                        qT[:, h, i0 + off:i0 + IT],
                                        start=True,
                                        stop=True,
                                    )
                                if jc0 + 1 < ndiag0:
                                    # non-diag pair: one exp over both banks
                                    nc.scalar.activation(
                                        Et2[:, :, :], ps2[:, :, :], EXP,
                                        scale=SCALE,
                                    )
                                    EtS = spool.tile([128, IT], F16,
                                                     tag="EtS")
                                    nc.vector.tensor_add(
                                        EtS[:], Et2[:, 0, :], Et2[:, 1, :]
                                    )
                                    if u % 2 == 1:
                                        if not st["t"]:
                                            nc.vector.tensor_add(
                                                T[:], st["es"][:], EtS[:]
                                            )
                                            st["t"] = True
                                        else:
                                            EtQ = spool.tile(
                                                [128, IT], F16, tag="EtQ")
                                            nc.vector.tensor_add(
                                                EtQ[:], st["es"][:], EtS[:]
                                            )
                                            nc.vector.tensor_add(
                                                T[:], T[:], EtQ[:]
                                            )
                                    st["es"] = EtS
                                    pending.append((jc0, Et2, 0, 0))
                                    pending.append((jc0 + 1, Et2, 1, 0))
                                else:
                                    # diag pair: ragged exps + mask + T
                                    for i in range(2):
                                        jc = jc0 + i
                                        off = 128 * (jc - ndiag0)
                                        nc.scalar.activation(
                                            Et2[:, i, off:],
                                            ps2[:, i, off:], EXP,
                                            scale=SCALE,
                                        )
                                        nc.vector.tensor_mul(
                                            Et2[:, i, off:off + 128],
                                            Et2[:, i, off:off + 128],
                                            triT[:],
                                        )
                                        if not st["t"]:
                                            nc.vector.tensor_copy(
                                                T[:], Et2[:, i, :]
                                            )
                                            st["t"] = True
                                        else:
                                            nc.vector.tensor_add(
                                                T[:, off:], T[:, off:],
                                                Et2[:, i, off:],
                                            )
                                        pending.append((jc, Et2, i, off))
                                while len(pending) > _T["PEND"]:
                                    emit_av(*pending.pop(0))
                            fill_slot(at_flush=True)
                            for item in pending:
                                emit_av(*item)
                            # rowsum across k (partitions) in one gpsimd op,
                            # broadcast to all partitions; then 1/sum on DVE
                            Rb = small.tile([128, IT], F32, tag="Rb")
                            nc.gpsimd.partition_all_reduce(
                                Rb[:], T[:], channels=128,
                                reduce_op=bass_isa.ReduceOp.add,
                            )
                            rec = small.tile([128, IT], F32, tag="rec")
                            nc.vector.reciprocal(rec[:], Rb[:])
                            Ocp = fpool.tile([128, IT], F32, tag="Ocp")
                            nc.vector.tensor_mul(Ocp[:], O[:], rec[:])
                            # defer the fp8 hi/lo split of this iteration's
                            # output until the next iteration, so Pool's
                            # all-reduce is never queued behind a hi-copy;
                            # the last head of each row-block flushes
                            # immediately so its y-pairs unlock before the
                            # next region's filler slots
                            flush_split()
                            deferred_split.append((Ocp, h, i0))
                            if h == HL - 1:
                                flush_split(on_dve=True)

                    # ---- phase 3: remaining output-projection groups ------
                    # batch stores two-pairs-at-a-time; the last two pairs
                    # split/quarter their stores so the post-PE drain is
                    # short
                    while ready_y:
                        if (len(ready_y) > 4
                                and ready_y[0][0] == ready_y[1][0]
                                and ready_y[0][1] == 0
                                and ready_y[1][1] == 1):
                            emit_y_quad()
                        else:
                            emit_y_pair(split_dma=False, in_ph2=False,
                                        final=(len(ready_y) == 1))
            xhold.release()
            whold.release()
    nc.compile()
    return nc


def _get_nc():
    if "nc" not in _CACHE:
        _CACHE["nc"] = _build()
    return _CACHE["nc"]


def _split8(a):
    hi = a.astype(NPF8)
    lo = (a - hi.astype(np.float32)).astype(NPF8)
    return hi, lo


def make_in_maps(x, Wq, Wk, Wv, Wo):
    x = np.asarray(x, np.float32)
    Wq = np.asarray(Wq, np.float32)
    Wk = np.asarray(Wk, np.float32)
    Wv = np.asarray(Wv, np.float32)
    Wo = np.asarray(Wo, np.float32)

    # keep-mask in [k_local (partition), q_local (col)] layout: keep k<=q
    tri01 = np.ascontiguousarray(np.triu(np.ones((128, 128), np.float16)))

    def _pack_x(xT8):
        # [E, S] -> [ei=128, tt, eo, TT] (phase-1 tile DMA order)
        return np.ascontiguousarray(
            xT8.reshape(EO, 128, S // TT, TT).transpose(1, 2, 0, 3)
        )

    xs = []
    for b in range(B):
        hi, lo = _split8(np.ascontiguousarray(x[b].T))
        xs.append((_pack_x(hi), _pack_x(lo)))
    def _pack_qk(w8):  # [E, F] -> [ei=128, fc, eo, 128]
        return np.ascontiguousarray(
            w8.reshape(EO, 128, HL, 128).transpose(1, 2, 0, 3)
        )

    def _pack_v(w8):  # [E, F] -> [ei=128, eo, F]
        return np.ascontiguousarray(
            w8.reshape(EO, 128, F).transpose(1, 0, 2)
        )

    def _pack_o(w8):  # [F, E] -> [fi=128, fc, E]
        return np.ascontiguousarray(
            w8.reshape(HL, 128, E).transpose(1, 0, 2)
        )

    in_maps = []
    for c in range(8):
        b, g = c // 4, c % 4
        fsl = slice(F * g, F * (g + 1))
        m = {"x1P": xs[b][0], "x2P": xs[b][1], "tri01": tri01}
        for n, W, pk in (("wq", Wq, _pack_qk), ("wk", Wk, _pack_qk),
                         ("wv", Wv, _pack_v)):
            hi, lo = _split8(np.ascontiguousarray(W[fsl, :].T) * WS)
            m[n + "1"], m[n + "2"] = pk(hi), pk(lo)
        hi, lo = _split8(np.ascontiguousarray(Wo[:, fsl].T) * WS)
        m["wo1"], m["wo2"] = _pack_o(hi), _pack_o(lo)
        in_maps.append(m)
    return in_maps


def combine_outputs(results):
    out = np.empty((B, S, E), np.float32)
    for b in range(B):
        acc = results[4 * b]["y"].astype(np.float32).copy()
        for g in range(1, 4):
            acc += results[4 * b + g]["y"]
        out[b] = acc
    return out


def kernel(x, Wq, Wk, Wv, Wo):
    import time as _time

    nc = _get_nc()
    in_maps = make_in_maps(x, Wq, Wk, Wv, Wo)
    last_exc = None
    for attempt in range(3):
        if attempt:
            # transient device wedge (e.g. NRT_EXEC_UNIT_UNRECOVERABLE) --
            # wait for recovery before retrying
            _time.sleep(30 * attempt)
        try:
            res = bass_utils.run_bass_kernel_spmd(
                nc, in_maps, core_ids=list(range(8))
            )
            return combine_outputs(res.results)
        except Exception as exc:
            last_exc = exc
    raise last_exc


# revision 67
# speedup vs baseline: 1.0509x; 1.0033x over previous
"""Multi-head attention (B=2, S=2048, E=2048, H=16, causal) on 8 TRN2 NeuronCores.

Sharding: 8 cores = 2 batch shards x 4 head-group shards (4 heads / 512
features each).  Each core runs the full attention stack for its (batch,
head-group) and produces a partial [S, E] output through its row-block of
Wo; the host sums the 4 partials per batch.

Projections (QKV, Wo) run as 3-term fp8e4m3 hi/lo split matmuls in
DoubleRow perf mode (A@B ~ A1B1 + A2B1 + A1B2, each term contracting
2x128 rows per pass at 0.5 cyc/row).  The hi/lo splits of x and the
weights are prepared on the host; weights are pre-scaled by 64 so their
values sit in fp8's normal range (the scale is undone on the way out).
Attention (scores, attn@v) runs in fp16 at full PE rate.

Scheduling structure:
- softmax rowsum and causal mask run OFF the PE: exp tiles are pre-summed
  on DVE (fp16 pair/quad tree) and reduced across partitions with one
  gpsimd partition_all_reduce per (head, query-tile); the diagonal
  128-block of each exp tile is masked by a 0/1 triangle multiply on DVE.
- score chunks are computed in PAIRS into 2-bank PSUM tiles so one Act
  exp instruction covers 1024 columns (halves Act instruction overhead --
  Act is the pacing engine during attention).
- projection chains whose outputs are consumed late (Q rows 0:512, used
  by the last-processed p=0 tile; K rows 1536:2048, used by p=3) are
  deferred out of phase 1 into phase 2, where they fill PE bubbles in the
  Act-paced attention stream.  Output-projection (Wo) chains fill the
  rest, paced per region.
"""

import os

import numpy as np

import concourse.bacc as bacc
import concourse.mybir as mybir
import concourse.tile as tile
from concourse import bass_isa
from concourse import bass_utils

_T = {
    "EPOOL": int(os.environ.get("K_EPOOL", "6")),
    "PEND": int(os.environ.get("K_PEND", "5")),
    "PSS": int(os.environ.get("K_PSS", "3")),
}

B, S, E, H = 2, 2048, 2048, 16
D = 128                    # head dim
HL = 4                     # heads per core
F = HL * D                 # local features = 512
EO = E // 128              # 16 contraction chunks
TT = 256                   # phase-1 token tile
IT = 512                   # phase-2 query tile
F32 = mybir.dt.float32
F16 = mybir.dt.float16
F8 = mybir.dt.float8e4
DR = mybir.MatmulPerfMode.DoubleRow
EXP = mybir.ActivationFunctionType.Exp
COPY = mybir.ActivationFunctionType.Copy
WS = 64.0                  # host-side weight prescale
SCALE = 1.0 / float(np.sqrt(D)) / (WS * WS)
YS = 1.0 / WS              # y de-scale: out1/2 unit scale, wo carries WS
NPF8 = mybir.dt.np(F8)

_CACHE = {}


def _build():
    nc = bacc.Bacc("TRN2", target_bir_lowering=False, debug=False)
    # x packed on host as [ei=128, tt, eo, TT] so each phase-1 tile loads as
    # one DMA with 4KB-contiguous per-partition payload (full bus efficiency)
    x1P = nc.dram_tensor("x1P", [128, S // TT, EO, TT], F8,
                         kind="ExternalInput").ap()
    x2P = nc.dram_tensor("x2P", [128, S // TT, EO, TT], F8,
                         kind="ExternalInput").ap()
    # all weights pre-packed on host into their SBUF layouts (contiguous
    # per-partition payloads -> full DMA bus efficiency); q/k are fc-major
    # so the first head-column can land in one small DMA at startup
    w_t = {
        n: nc.dram_tensor(n, [128, HL, EO, 128], F8, kind="ExternalInput").ap()
        for n in ("wq1", "wq2", "wk1", "wk2")
    }
    for n in ("wv1", "wv2"):
        w_t[n] = nc.dram_tensor(n, [128, EO, F], F8, kind="ExternalInput").ap()
    wo1_t = nc.dram_tensor("wo1", [128, HL, E], F8, kind="ExternalInput").ap()
    wo2_t = nc.dram_tensor("wo2", [128, HL, E], F8, kind="ExternalInput").ap()
    # 0/1 keep-mask for the k>q half of a diagonal scores block (DVE)
    tri_t = nc.dram_tensor("tri01", [128, 128], F16, kind="ExternalInput").ap()
    y = nc.dram_tensor("y", [S, E], mybir.dt.bfloat16, kind="ExternalOutput").ap()

    with tile.TileContext(nc) as tc:
        with tc.tile_pool(name="persist", bufs=1) as persist:
            qT = persist.tile([128, HL, S], F16, tag="qT")
            kT = persist.tile([128, HL, S], F16, tag="kT")
            vN = persist.tile([128, S // 128, F], F16, tag="vN")
            out1 = persist.tile([128, HL, S], F8, tag="out1")
            out2 = persist.tile([128, HL, S], F8, tag="out2")
            triT = persist.tile([128, 128], F16, tag="triT")

            # wq/wk + four x tiles persist into phase 2 for the deferred
            # projection chains (Q rows 0:512 -> consumed by p=0 last;
            # K rows 1536:2048 -> consumed by p=3)
            whold = tc.alloc_tile_pool(name="whold", bufs=1)
            xhold = tc.alloc_tile_pool(name="xhold", bufs=1)
            wres = {}
            for n in ("wq1", "wq2", "wk1", "wk2"):
                wres[n] = whold.tile([128, HL, EO, 128], F8, tag=n, name=n)
            xt = {}
            for tt in (0, 1, 6, 7):
                xt[tt] = (
                    xhold.tile([128, EO, TT], F8, tag=f"x1h{tt}",
                               name=f"x1h{tt}"),
                    xhold.tile([128, EO, TT], F8, tag=f"x2h{tt}",
                               name=f"x2h{tt}"),
                )

            # ---------- phase 1: q/k/v projections ------------------------
            with (
                tc.tile_pool(name="wres", bufs=1) as wpool,
                tc.tile_pool(name="xstream", bufs=4) as xpool,
                tc.tile_pool(name="ps_qk", bufs=5, space="PSUM") as ps_qk,
                tc.tile_pool(name="ps_v", bufs=2, space="PSUM") as ps_v,
            ):
                for n in ("wv1", "wv2"):
                    wres[n] = wpool.tile([128, EO, F], F8, tag=n, name=n)
                # startup order: the first K chain's inputs lead (wq is
                # deferred, so K/V data owns the bus); wv streams in
                # chunk-quarters interleaved with the later wk columns so
                # the tt=0 V chains aren't starved behind 1MB transfers
                nc.sync.dma_start(wres["wk1"][:, 0], w_t["wk1"][:, 0])
                nc.sync.dma_start(xt[0][0][:, 0:8], x1P[:, 0, 0:8])
                nc.sync.dma_start(xt[0][0][:, 8:], x1P[:, 0, 8:])
                nc.sync.dma_start(xt[0][1][:, 0:8], x2P[:, 0, 0:8])
                nc.sync.dma_start(xt[0][1][:, 8:], x2P[:, 0, 8:])
                nc.sync.dma_start(wres["wk2"][:, 0], w_t["wk2"][:, 0])
                nc.sync.dma_start(wres["wk1"][:, 1], w_t["wk1"][:, 1])
                nc.sync.dma_start(wres["wk2"][:, 1], w_t["wk2"][:, 1])
                nc.sync.dma_start(wres["wv1"][:, 0:4], w_t["wv1"][:, 0:4])
                nc.sync.dma_start(wres["wk1"][:, 2], w_t["wk1"][:, 2])
                nc.sync.dma_start(wres["wk2"][:, 2], w_t["wk2"][:, 2])
                nc.sync.dma_start(wres["wv1"][:, 4:8], w_t["wv1"][:, 4:8])
                nc.sync.dma_start(wres["wk1"][:, 3], w_t["wk1"][:, 3])
                nc.sync.dma_start(wres["wk2"][:, 3], w_t["wk2"][:, 3])
                nc.sync.dma_start(wres["wv1"][:, 8:], w_t["wv1"][:, 8:])
                nc.sync.dma_start(wres["wv2"][:, 0:8], w_t["wv2"][:, 0:8])
                nc.sync.dma_start(wres["wv2"][:, 8:], w_t["wv2"][:, 8:])
                nc.sync.dma_start(xt[1][0][:], x1P[:, 1])
                nc.sync.dma_start(xt[1][1][:], x2P[:, 1])
                nc.sync.dma_start(wres["wq1"][:], w_t["wq1"])
                nc.sync.dma_start(wres["wq2"][:], w_t["wq2"])
                deferred = []   # K chains first (earlier deadline), then Q
                for tt in range(S // TT):
                    t0 = tt * TT
                    if tt in xt:
                        x1, x2 = xt[tt]
                        if tt >= 6:
                            nc.sync.dma_start(x1[:], x1P[:, tt])
                            nc.sync.dma_start(x2[:], x2P[:, tt])
                    else:
                        x1 = xpool.tile([128, EO, TT], F8, tag="x1")
                        x2 = xpool.tile([128, EO, TT], F8, tag="x2")
                        nc.sync.dma_start(x1[:], x1P[:, tt])
                        nc.sync.dma_start(x2[:], x2P[:, tt])
                    if tt == 4:
                        nc.sync.dma_start(triT[:], tri_t)
                    def emit_qk(x1, x2, t0, tt):
                        for wn, dst in (("wq", qT), ("wk", kT)):
                            if wn == "wq" and tt < 2:
                                for fc in range(HL):
                                    deferred.append(("wq", qT, fc, x1, x2,
                                                     t0))
                                continue
                            if wn == "wk" and tt >= 6:
                                for fc in range(HL):
                                    deferred.append(("wk", kT, fc, x1, x2,
                                                     t0))
                                continue
                            w1, w2 = wres[wn + "1"], wres[wn + "2"]
                            for fc in range(HL):
                                ps = ps_qk.tile([128, TT], F32, tag="pqk")
                                terms = (
                                    [(w1, x1, g) for g in range(0, EO, 2)]
                                    + [(w1, x2, g) for g in range(0, EO, 2)]
                                    + [(w2, x1, g) for g in range(0, EO, 2)]
                                )
                                for i, (w, x, g) in enumerate(terms):
                                    nc.tensor.matmul(
                                        ps[:],
                                        w[:, fc, g:g + 2, :],
                                        x[:, g:g + 2, :],
                                        start=(i == 0),
                                        stop=(i == len(terms) - 1),
                                        perf_mode=DR,
                                    )
                                nc.vector.tensor_copy(
                                    dst[:, fc, t0:t0 + TT], ps[:]
                                )

                    def emit_v(x1, x2, t0):
                        w1, w2 = wres["wv1"], wres["wv2"]
                        for tc2 in range(TT // 128):
                            tsl = slice(tc2 * 128, (tc2 + 1) * 128)
                            ps = ps_v.tile([128, F], F32, tag="pv")
                            terms = (
                                [(x1, w1, g) for g in range(0, EO, 2)]
                                + [(x2, w1, g) for g in range(0, EO, 2)]
                                + [(x1, w2, g) for g in range(0, EO, 2)]
                            )
                            for i, (x, w, g) in enumerate(terms):
                                nc.tensor.matmul(
                                    ps[:],
                                    x[:, g:g + 2, tsl],
                                    w[:, g:g + 2, :],
                                    start=(i == 0),
                                    stop=(i == len(terms) - 1),
                                    perf_mode=DR,
                                )
                            # v lands at unit scale (the wv prescale is
                            # undone here) so the normalized attention
                            # output fits fp8e4's range for the hi/lo split
                            nc.vector.tensor_scalar_mul(
                                vN[:, (t0 // 128) + tc2, :], ps[:], 1.0 / WS
                            )

                    if tt == S // TT - 1:
                        # last tile: V first, so its slow PSUM->SBUF drain
                        # isn't the phase-boundary gate for the reallocated
                        # phase-2 banks
                        emit_v(x1, x2, t0)
                        emit_qk(x1, x2, t0, tt)
                    else:
                        emit_qk(x1, x2, t0, tt)
                        emit_v(x1, x2, t0)
                deferred.sort(key=lambda d: d[0] != "wk")

            # ---------- phase 2: attention per head ----------------------
            with tc.tile_pool(name="wo", bufs=1) as wo_pool:
                wo1_r = wo_pool.tile([128, HL, E], F8, tag="wo1")
                wo2_r = wo_pool.tile([128, HL, E], F8, tag="wo2")
                nc.sync.dma_start(wo1_r[:], wo1_t)
                nc.sync.dma_start(wo2_r[:], wo2_t)

                with (
                    tc.tile_pool(name="ph2", bufs=_T["EPOOL"]) as epool,
                    tc.tile_pool(name="ph2s", bufs=3) as spool,
                    tc.tile_pool(name="ph2t", bufs=2) as tpool,
                    tc.tile_pool(name="ph2b", bufs=2) as small,
                    tc.tile_pool(name="ph2f", bufs=3) as fpool,
                    tc.tile_pool(name="ps_s", bufs=_T["PSS"], space="PSUM") as ps_s,
                    tc.tile_pool(name="ps_o", bufs=2, space="PSUM") as ps_o,
                    tc.tile_pool(name="ystb", bufs=4) as ystb_pool,
                ):
                    ready_y = []

                    dq_state = {"cur": None, "idx": 0}

                    def emit_deferred_terms(nterms):
                        # deferred projection chains emitted in term-granular
                        # slices: fine-grained PE filler that matches the
                        # small per-pair Act deficit instead of overshooting
                        while nterms > 0:
                            if dq_state["cur"] is None:
                                if not deferred:
                                    return
                                wn, dst, fc, x1, x2, t0 = deferred.pop(0)
                                w1, w2 = wres[wn + "1"], wres[wn + "2"]
                                pst = ps_s.tile([128, 2, 512], F32,
                                                tag="S")
                                ps = pst[:, 0, 0:TT]
                                terms = (
                                    [(w1, x1, g) for g in range(0, EO, 2)]
                                    + [(w1, x2, g) for g in range(0, EO, 2)]
                                    + [(w2, x1, g) for g in range(0, EO, 2)]
                                )
                                dq_state["cur"] = (dst, fc, t0, ps, terms)
                                dq_state["idx"] = 0
                            dst, fc, t0, ps, terms = dq_state["cur"]
                            i0x = dq_state["idx"]
                            take = min(nterms, len(terms) - i0x)
                            for i in range(i0x, i0x + take):
                                w, x, g = terms[i]
                                nc.tensor.matmul(
                                    ps,
                                    w[:, fc, g:g + 2, :],
                                    x[:, g:g + 2, :],
                                    start=(i == 0),
                                    stop=(i == len(terms) - 1),
                                    perf_mode=DR,
                                )
                            dq_state["idx"] += take
                            nterms -= take
                            if dq_state["idx"] == len(terms):
                                nc.vector.tensor_copy(
                                    dst[:, fc, t0:t0 + TT], ps
                                )
                                dq_state["cur"] = None

                    def emit_y_pair(split_dma=False, in_ph2=True,
                                    final=False):
                        # one (tcb, et-pair) group: two Wo psum chains into a
                        # single [128, 1024] bf16 store
                        tcb, ep = ready_y.pop(0)
                        tsl = slice(tcb * 128, (tcb + 1) * 128)
                        yb = ystb_pool.tile([128, 1024], mybir.dt.bfloat16,
                                            tag="yb")
                        if in_ph2:
                            # both chains share one scores-pair tile (a
                            # half each): no dedicated y bank, so scores
                            # keep a 3-buffer rotation
                            Ypair = ps_s.tile([128, 2, 512], F32, tag="S")
                        for j in range(2):
                            esl = slice((2 * ep + j) * 512,
                                        (2 * ep + j + 1) * 512)
                            if in_ph2:
                                Ysl = (lambda jj: lambda a, b:
                                       Ypair[:, jj, a:b])(j)
                            else:
                                # phase 3: the attention O banks are free
                                Ybt = ps_o.tile([128, IT], F32, tag="O")
                                Ysl = (lambda t: lambda a, b: t[:, a:b])(Ybt)
                            Yb = Ysl(0, 512)
                            terms = []
                            for fp in range(HL // 2):
                                g = 2 * fp
                                terms += [(out1, wo1_r, g), (out2, wo1_r, g),
                                          (out1, wo2_r, g)]
                            for i, (o, w, g) in enumerate(terms):
                                nc.tensor.matmul(
                                    Yb,
                                    o[:, g:g + 2, tsl],
                                    w[:, g:g + 2, esl],
                                    start=(i == 0),
                                    stop=(i == len(terms) - 1),
                                    perf_mode=DR,
                                )
                            if final and j == 1:
                                # last store of the kernel: one Act scale,
                                # one [128,512] store -- every extra store
                                # costs a serial HWDGE slot (625ns), so no
                                # further splitting pays off
                                nc.scalar.activation(
                                    yb[:, 512:1024], Yb, COPY, scale=YS,
                                )
                                nc.sync.dma_start(y[tsl, esl],
                                                  yb[:, 512:1024])
                                continue
                            if j == 0:
                                # Act: keeps the single ps_yb bank's drain
                                # off DVE's in-order queue
                                nc.scalar.activation(
                                    yb[:, 0:512], Yb, COPY, scale=YS,
                                )
                            else:
                                # DVE (gpsimd cannot read PSUM)
                                nc.vector.tensor_scalar_mul(
                                    yb[:, 512:1024], Yb, YS
                                )
                            if split_dma or final:
                                nc.sync.dma_start(y[tsl, esl],
                                                  yb[:, j * 512:(j + 1) * 512])
                        if not (split_dma or final):
                            nc.sync.dma_start(
                                y[tsl, ep * 1024:(ep + 1) * 1024], yb[:]
                            )

                    def emit_y_quad():
                        # phase 3 only: two adjacent (tcb, ep=0/1) groups,
                        # four Wo chains, ONE [128, 2048] store -- halves the
                        # per-store issue overhead (SP config + HWDGE are a
                        # shared serial resource that otherwise can't keep
                        # pace with the chain cadence)
                        tcb, _ = ready_y.pop(0)
                        ready_y.pop(0)
                        tsl = slice(tcb * 128, (tcb + 1) * 128)
                        yb2 = ystb_pool.tile([128, 2048], mybir.dt.bfloat16,
                                             tag="yb2")
                        for half in range(4):
                            esl = slice(half * 512, (half + 1) * 512)
                            if half % 2 == 0:
                                Ybt = ps_o.tile([128, IT], F32, tag="O")
                                Yb = Ybt[:]
                            else:
                                Ybt = ps_s.tile([128, 2, 512], F32, tag="S")
                                Yb = Ybt[:, 0, :]
                            terms = []
                            for fp in range(HL // 2):
                                g = 2 * fp
                                terms += [(out1, wo1_r, g), (out2, wo1_r, g),
                                          (out1, wo2_r, g)]
                            for i, (o, w, g) in enumerate(terms):
                                nc.tensor.matmul(
                                    Yb,
                                    o[:, g:g + 2, tsl],
                                    w[:, g:g + 2, esl],
                                    start=(i == 0),
                                    stop=(i == len(terms) - 1),
                                    perf_mode=DR,
                                )
                            if half % 2 == 0:
                                nc.scalar.activation(
                                    yb2[:, esl], Yb, COPY, scale=YS,
                                )
                            else:
                                nc.vector.tensor_scalar_mul(
                                    yb2[:, esl], Yb, YS
                                )
                        nc.sync.dma_start(y[tsl, :], yb2[:])

                    deferred_split = []

                    def flush_split(on_dve=False):
                        while deferred_split:
                            Ocp_d, h_d, i0_d = deferred_split.pop(0)
                            # hi/lo split on Pool: keeps the per-head
                            # normalize chain off DVE's in-order queue.
                            # The release-critical last head of a row-block
                            # splits on DVE instead (shorter chain -> its
                            # y-pairs unlock sooner)
                            eng = nc.vector if on_dve else nc.gpsimd
                            eng.tensor_copy(
                                out1[:, h_d, i0_d:i0_d + IT], Ocp_d[:]
                            )
                            eng.tensor_sub(
                                out2[:, h_d, i0_d:i0_d + IT], Ocp_d[:],
                                out1[:, h_d, i0_d:i0_d + IT],
                            )
                            if h_d == HL - 1:
                                # the head-group's outputs are now all
                                # written -- its y pairs may be emitted
                                pd = i0_d // IT
                                for tcb_r in range(4 * pd, 4 * pd + 4):
                                    for ep_r in range(E // 1024):
                                        ready_y.append((tcb_r, ep_r))

                    # p0 (shortest, latency-bound) runs last, when y-pair
                    # chains exist to fill PE while Act/Pool/DVE drain
                    p_order = (1, 2, 0, 3)
                    for pi, p in enumerate(p_order):
                        i0 = p * IT
                        for h in range(HL):
                            h0 = h * 128
                            njc = (i0 + IT) // 128
                            npair = njc // 2
                            ndiag0 = i0 // 128   # first diag chunk index
                            O = ps_o.tile([128, IT], F32, tag="O")
                            # fp16 running rowsum accumulator; reduced
                            # across partitions at the end by one gpsimd
                            # all-reduce
                            T = tpool.tile([128, IT], F16, tag="T")

                            pending = []
                            st = {"t": False, "es": None}

                            def fill_slot(u=None, at_flush=False):
                                # PE filler: term-granular deferred chains
                                # in the Act-paced early regions, Wo chains
                                # later; p=0 fills only after its exps (so
                                # Act's in-order queue stays on exp) and
                                # keeps 2 pairs back to bridge into phase 3
                                if p == 0:
                                    if len(ready_y) > 3:
                                        emit_y_pair()
                                elif p == 3:
                                    if (not at_flush and u in (1, npair - 1)
                                            and len(ready_y) > 4):
                                        emit_y_pair()
                                    elif at_flush and len(ready_y) > 6:
                                        emit_y_pair()
                                    elif not at_flush and u == 5:
                                        emit_deferred_terms(24)
                                elif p == 2:
                                    if at_flush and len(ready_y) > 6:
                                        emit_y_pair()
                                    elif not at_flush and u in (1, 3):
                                        emit_deferred_terms(24)
                                else:
                                    if not at_flush and u in (1, 3):
                                        emit_deferred_terms(24)

                            def emit_av(jc, Et2, i, off):
                                nc.tensor.matmul(
                                    O[:, off:],
                                    vN[:, jc, h0:h0 + 128],
                                    Et2[:, i, off:],
                                    start=(jc == 0),
                                    stop=(jc == njc - 1),
                                )

                            for u in range(npair):
                                jc0 = 2 * u
                                fill_slot(u=u)
                                # --- scores pair ---------------------------
                                ps2 = ps_s.tile([128, 2, 512], F32, tag="S")
                                Et2 = epool.tile([128, 2, 512], F16, tag="E2")
                                for i in range(2):
                                    jc = jc0 + i
                                    q_off = jc - ndiag0
                                    off = 0 if q_off < 0 else 128 * q_off
                                    nc.tensor.matmul(
                                        ps2[:, i, off:],
                                        kT[:, h, jc * 128:(jc + 1) * 128],
                                        qT[:, h, i0 + off:i0 + IT],
                                        start=True,
                                        stop=True,
                                    )
                                if jc0 + 1 < ndiag0:
                                    # non-diag pair: one exp over both banks
                                    nc.scalar.activation(
                                        Et2[:, :, :], ps2[:, :, :], EXP,
                                        scale=SCALE,
                                    )
                                    EtS = spool.tile([128, IT], F16,
                                                     tag="EtS")
                                    nc.vector.tensor_add(
                                        EtS[:], Et2[:, 0, :], Et2[:, 1, :]
                                    )
                                    if u % 2 == 1:
                                        if not st["t"]:
                                            nc.vector.tensor_add(
                                                T[:], st["es"][:], EtS[:]
                                            )
                                            st["t"] = True
                                        else:
                                            EtQ = spool.tile(
                                                [128, IT], F16, tag="EtQ")
                                            nc.vector.tensor_add(
                                                EtQ[:], st["es"][:], EtS[:]
                                            )
                                            nc.vector.tensor_add(
                                                T[:], T[:], EtQ[:]
                                            )
                                    st["es"] = EtS
                                    pending.append((jc0, Et2, 0, 0))
                                    pending.append((jc0 + 1, Et2, 1, 0))
                                else:
                                    # diag pair: ragged exps + mask + T
                                    for i in range(2):
                                        jc = jc0 + i
                                        off = 128 * (jc - ndiag0)
                                        nc.scalar.activation(
                                            Et2[:, i, off:],
                                            ps2[:, i, off:], EXP,
                                            scale=SCALE,
                                        )
                                        nc.vector.tensor_mul(
                                            Et2[:, i, off:off + 128],
                                            Et2[:, i, off:off + 128],
                                            triT[:],
                                        )
                                        if not st["t"]:
                                            nc.vector.tensor_copy(
                                                T[:], Et2[:, i, :]
                                            )
                                            st["t"] = True
                                        else:
                                            nc.vector.tensor_add(
                                                T[:, off:], T[:, off:],
                                                Et2[:, i, off:],
                                            )
                                        pending.append((jc, Et2, i, off))
                                while len(pending) > _T["PEND"]:
                                    emit_av(*pending.pop(0))
                            fill_slot(at_flush=True)
                            for item in pending:
                                emit_av(*item)
                            # rowsum across k (partitions) in one gpsimd op,
                            # broadcast to all partitions; then 1/sum on DVE
                            Rb = small.tile([128, IT], F32, tag="Rb")
                            nc.gpsimd.partition_all_reduce(
                                Rb[:], T[:], channels=128,
                                reduce_op=bass_isa.ReduceOp.add,
                            )
                            rec = small.tile([128, IT], F32, tag="rec")
                            nc.vector.reciprocal(rec[:], Rb[:])
                            Ocp = fpool.tile([128, IT], F32, tag="Ocp")
                            nc.vector.tensor_mul(Ocp[:], O[:], rec[:])
                            # defer the fp8 hi/lo split of this iteration's
                            # output until the next iteration, so Pool's
                            # all-reduce is never queued behind a hi-copy;
                            # the last head of each row-block flushes
                            # immediately so its y-pairs unlock before the
                            # next region's filler slots
                            flush_split()
                            deferred_split.append((Ocp, h, i0))
                            if h == HL - 1:
                                flush_split(on_dve=True)

                    # ---- phase 3: remaining output-projection groups ------
                    # batch stores two-pairs-at-a-time; the last two pairs
                    # split/quarter their stores so the post-PE drain is
                    # short
                    while ready_y:
                        if (len(ready_y) > 4
                                and ready_y[0][0] == ready_y[1][0]
                                and ready_y[0][1] == 0
                                and ready_y[1][1] == 1):
                            emit_y_quad()
                        else:
                            emit_y_pair(split_dma=False, in_ph2=False,
                                        final=(len(ready_y) == 1))
            xhold.release()
            whold.release()
    nc.compile()
    return nc


def _get_nc():
    if "nc" not in _CACHE:
        _CACHE["nc"] = _build()
    return _CACHE["nc"]


def _split8(a):
    hi = a.astype(NPF8)
    lo = (a - hi.astype(np.float32)).astype(NPF8)
    return hi, lo


def make_in_maps(x, Wq, Wk, Wv, Wo):
    x = np.asarray(x, np.float32)
    Wq = np.asarray(Wq, np.float32)
    Wk = np.asarray(Wk, np.float32)
    Wv = np.asarray(Wv, np.float32)
    Wo = np.asarray(Wo, np.float32)

    # keep-mask in [k_local (partition), q_local (col)] layout: keep k<=q
    tri01 = np.ascontiguousarray(np.triu(np.ones((128, 128), np.float16)))

    def _pack_x(xT8):
        # [E, S] -> [ei=128, tt, eo, TT] (phase-1 tile DMA order)
        return np.ascontiguousarray(
            xT8.reshape(EO, 128, S // TT, TT).transpose(1, 2, 0, 3)
        )

    xs = []
    for b in range(B):
        hi, lo = _split8(np.ascontiguousarray(x[b].T))
        xs.append((_pack_x(hi), _pack_x(lo)))
    def _pack_qk(w8):  # [E, F] -> [ei=128, fc, eo, 128]
        return np.ascontiguousarray(
            w8.reshape(EO, 128, HL, 128).transpose(1, 2, 0, 3)
        )

    def _pack_v(w8):  # [E, F] -> [ei=128, eo, F]
        return np.ascontiguousarray(
            w8.reshape(EO, 128, F).transpose(1, 0, 2)
        )

    def _pack_o(w8):  # [F, E] -> [fi=128, fc, E]
        return np.ascontiguousarray(
            w8.reshape(HL, 128, E).transpose(1, 0, 2)
        )

    in_maps = []
    for c in range(8):
        b, g = c // 4, c % 4
        fsl = slice(F * g, F * (g + 1))
        m = {"x1P": xs[b][0], "x2P": xs[b][1], "tri01": tri01}
        for n, W, pk in (("wq", Wq, _pack_qk), ("wk", Wk, _pack_qk),
                         ("wv", Wv, _pack_v)):
            hi, lo = _split8(np.ascontiguousarray(W[fsl, :].T) * WS)
            m[n + "1"], m[n + "2"] = pk(hi), pk(lo)
        hi, lo = _split8(np.ascontiguousarray(Wo[:, fsl].T) * WS)
        m["wo1"], m["wo2"] = _pack_o(hi), _pack_o(lo)
        in_maps.append(m)
    return in_maps


def combine_outputs(results):
    out = np.empty((B, S, E), np.float32)
    for b in range(B):
        acc = results[4 * b]["y"].astype(np.float32).copy()
        for g in range(1, 4):
            acc += results[4 * b + g]["y"]
        out[b] = acc
    return out


def kernel(x, Wq, Wk, Wv, Wo):
    import time as _time

    nc = _get_nc()
    in_maps = make_in_maps(x, Wq, Wk, Wv, Wo)
    last_exc = None
    for attempt in range(3):
        if attempt:
            # transient device wedge (e.g. NRT_EXEC_UNIT_UNRECOVERABLE) --
            # wait for recovery before retrying
            _time.sleep(30 * attempt)
        try:
            res = bass_utils.run_bass_kernel_spmd(
                nc, in_maps, core_ids=list(range(8))
            )
            return combine_outputs(res.results)
        except Exception as exc:
            last_exc = exc
    raise last_exc


# revision 73
# speedup vs baseline: 1.0529x; 1.0019x over previous
"""Multi-head attention (B=2, S=2048, E=2048, H=16, causal) on 8 TRN2 NeuronCores.

Sharding: 8 cores = 2 batch shards x 4 head-group shards (4 heads / 512
features each).  Each core runs the full attention stack for its (batch,
head-group) and produces a partial [S, E] output through its row-block of
Wo; the host sums the 4 partials per batch.

Projections (QKV, Wo) run as 3-term fp8e4m3 hi/lo split matmuls in
DoubleRow perf mode (A@B ~ A1B1 + A2B1 + A1B2, each term contracting
2x128 rows per pass at 0.5 cyc/row).  The hi/lo splits of x and the
weights are prepared on the host; weights are pre-scaled by 64 so their
values sit in fp8's normal range (the scale is undone on the way out).
Attention (scores, attn@v) runs in fp16 at full PE rate.

Scheduling structure:
- softmax rowsum and causal mask run OFF the PE: exp tiles are pre-summed
  on DVE (fp16 pair/quad tree) and reduced across partitions with one
  gpsimd partition_all_reduce per (head, query-tile); the diagonal
  128-block of each exp tile is masked by a 0/1 triangle multiply on DVE.
- score chunks are computed in PAIRS into 2-bank PSUM tiles so one Act
  exp instruction covers 1024 columns (halves Act instruction overhead --
  Act is the pacing engine during attention).
- projection chains whose outputs are consumed late (Q rows 0:512, used
  by the last-processed p=0 tile; K rows 1536:2048, used by p=3) are
  deferred out of phase 1 into phase 2, where they fill PE bubbles in the
  Act-paced attention stream.  Output-projection (Wo) chains fill the
  rest, paced per region.
"""

import os

import numpy as np

import concourse.bacc as bacc
import concourse.mybir as mybir
import concourse.tile as tile
from concourse import bass_isa
from concourse import bass_utils

_T = {
    "EPOOL": int(os.environ.get("K_EPOOL", "6")),
    "PEND": int(os.environ.get("K_PEND", "5")),
    "PSS": int(os.environ.get("K_PSS", "3")),
}

B, S, E, H = 2, 2048, 2048, 16
D = 128                    # head dim
HL = 4                     # heads per core
F = HL * D                 # local features = 512
EO = E // 128              # 16 contraction chunks
TT = 256                   # phase-1 token tile
IT = 512                   # phase-2 query tile
F32 = mybir.dt.float32
F16 = mybir.dt.float16
F8 = mybir.dt.float8e4
DR = mybir.MatmulPerfMode.DoubleRow
EXP = mybir.ActivationFunctionType.Exp
COPY = mybir.ActivationFunctionType.Copy
WS = 64.0                  # host-side weight prescale
SCALE = 1.0 / float(np.sqrt(D)) / (WS * WS)
YS = 1.0 / WS              # y de-scale: out1/2 unit scale, wo carries WS
NPF8 = mybir.dt.np(F8)

_CACHE = {}


def _build():
    nc = bacc.Bacc("TRN2", target_bir_lowering=False, debug=False)
    # x packed on host as [ei=128, tt, eo, TT] so each phase-1 tile loads as
    # one DMA with 4KB-contiguous per-partition payload (full bus efficiency)
    x1P = nc.dram_tensor("x1P", [128, S // TT, EO, TT], F8,
                         kind="ExternalInput").ap()
    x2P = nc.dram_tensor("x2P", [128, S // TT, EO, TT], F8,
                         kind="ExternalInput").ap()
    # all weights pre-packed on host into their SBUF layouts (contiguous
    # per-partition payloads -> full DMA bus efficiency); q/k are fc-major
    # so the first head-column can land in one small DMA at startup
    w_t = {
        n: nc.dram_tensor(n, [128, HL, EO, 128], F8, kind="ExternalInput").ap()
        for n in ("wq1", "wq2", "wk1", "wk2")
    }
    for n in ("wv1", "wv2"):
        w_t[n] = nc.dram_tensor(n, [128, EO, F], F8, kind="ExternalInput").ap()
    wo1_t = nc.dram_tensor("wo1", [128, HL, E], F8, kind="ExternalInput").ap()
    wo2_t = nc.dram_tensor("wo2", [128, HL, E], F8, kind="ExternalInput").ap()
    # 0/1 keep-mask for the k>q half of a diagonal scores block (DVE)
    tri_t = nc.dram_tensor("tri01", [128, 128], F16, kind="ExternalInput").ap()
    y = nc.dram_tensor("y", [S, E], mybir.dt.bfloat16, kind="ExternalOutput").ap()

    with tile.TileContext(nc) as tc:
        with tc.tile_pool(name="persist", bufs=1) as persist:
            qT = persist.tile([128, HL, S], F16, tag="qT")
            kT = persist.tile([128, HL, S], F16, tag="kT")
            vN = persist.tile([128, S // 128, F], F16, tag="vN")
            out1 = persist.tile([128, HL, S], F8, tag="out1")
            out2 = persist.tile([128, HL, S], F8, tag="out2")
            triT = persist.tile([128, 128], F16, tag="triT")

            # wq/wk + four x tiles persist into phase 2 for the deferred
            # projection chains (Q rows 0:512 -> consumed by p=0 last;
            # K rows 1536:2048 -> consumed by p=3)
            whold = tc.alloc_tile_pool(name="whold", bufs=1)
            xhold = tc.alloc_tile_pool(name="xhold", bufs=1)
            wres = {}
            for n in ("wq1", "wq2", "wk1", "wk2"):
                wres[n] = whold.tile([128, HL, EO, 128], F8, tag=n, name=n)
            xt = {}
            for tt in (0, 1, 6, 7):
                xt[tt] = (
                    xhold.tile([128, EO, TT], F8, tag=f"x1h{tt}",
                               name=f"x1h{tt}"),
                    xhold.tile([128, EO, TT], F8, tag=f"x2h{tt}",
                               name=f"x2h{tt}"),
                )

            # ---------- phase 1: q/k/v projections ------------------------
            with (
                tc.tile_pool(name="wres", bufs=1) as wpool,
                tc.tile_pool(name="xstream", bufs=4) as xpool,
                tc.tile_pool(name="ps_qk", bufs=5, space="PSUM") as ps_qk,
                tc.tile_pool(name="ps_v", bufs=2, space="PSUM") as ps_v,
            ):
                for n in ("wv1", "wv2"):
                    wres[n] = wpool.tile([128, EO, F], F8, tag=n, name=n)
                # startup order: the first K chain's inputs lead (wq is
                # deferred, so K/V data owns the bus); wv streams in
                # chunk-quarters interleaved with the later wk columns so
                # the tt=0 V chains aren't starved behind 1MB transfers
                nc.sync.dma_start(wres["wk1"][:, 0], w_t["wk1"][:, 0])
                nc.sync.dma_start(xt[0][0][:, 0:8], x1P[:, 0, 0:8])
                nc.sync.dma_start(xt[0][0][:, 8:], x1P[:, 0, 8:])
                nc.sync.dma_start(xt[0][1][:, 0:8], x2P[:, 0, 0:8])
                nc.sync.dma_start(xt[0][1][:, 8:], x2P[:, 0, 8:])
                nc.sync.dma_start(wres["wk2"][:, 0], w_t["wk2"][:, 0])
                nc.sync.dma_start(wres["wk1"][:, 1], w_t["wk1"][:, 1])
                nc.sync.dma_start(wres["wk2"][:, 1], w_t["wk2"][:, 1])
                nc.sync.dma_start(wres["wv1"][:, 0:4], w_t["wv1"][:, 0:4])
                nc.sync.dma_start(wres["wk1"][:, 2], w_t["wk1"][:, 2])
                nc.sync.dma_start(wres["wk2"][:, 2], w_t["wk2"][:, 2])
                nc.sync.dma_start(wres["wv1"][:, 4:8], w_t["wv1"][:, 4:8])
                nc.sync.dma_start(wres["wk1"][:, 3], w_t["wk1"][:, 3])
                nc.sync.dma_start(wres["wk2"][:, 3], w_t["wk2"][:, 3])
                nc.sync.dma_start(wres["wv1"][:, 8:], w_t["wv1"][:, 8:])
                nc.sync.dma_start(wres["wv2"][:, 0:8], w_t["wv2"][:, 0:8])
                nc.sync.dma_start(wres["wv2"][:, 8:], w_t["wv2"][:, 8:])
                nc.sync.dma_start(xt[1][0][:], x1P[:, 1])
                nc.sync.dma_start(xt[1][1][:], x2P[:, 1])
                nc.sync.dma_start(wres["wq1"][:], w_t["wq1"])
                nc.sync.dma_start(wres["wq2"][:], w_t["wq2"])
                deferred = []   # K chains first (earlier deadline), then Q
                for tt in range(S // TT):
                    t0 = tt * TT
                    if tt in xt:
                        x1, x2 = xt[tt]
                        if tt >= 6:
                            nc.sync.dma_start(x1[:], x1P[:, tt])
                            nc.sync.dma_start(x2[:], x2P[:, tt])
                    else:
                        x1 = xpool.tile([128, EO, TT], F8, tag="x1")
                        x2 = xpool.tile([128, EO, TT], F8, tag="x2")
                        nc.sync.dma_start(x1[:], x1P[:, tt])
                        nc.sync.dma_start(x2[:], x2P[:, tt])
                    if tt == 4:
                        nc.sync.dma_start(triT[:], tri_t)
                    def emit_qk(x1, x2, t0, tt):
                        for wn, dst in (("wq", qT), ("wk", kT)):
                            if wn == "wq" and tt < 2:
                                for fc in range(HL):
                                    deferred.append(("wq", qT, fc, x1, x2,
                                                     t0))
                                continue
                            if wn == "wk" and tt >= 6:
                                for fc in range(HL):
                                    deferred.append(("wk", kT, fc, x1, x2,
                                                     t0))
                                continue
                            w1, w2 = wres[wn + "1"], wres[wn + "2"]
                            for fc in range(HL):
                                ps = ps_qk.tile([128, TT], F32, tag="pqk")
                                terms = (
                                    [(w1, x1, g) for g in range(0, EO, 2)]
                                    + [(w1, x2, g) for g in range(0, EO, 2)]
                                    + [(w2, x1, g) for g in range(0, EO, 2)]
                                )
                                for i, (w, x, g) in enumerate(terms):
                                    nc.tensor.matmul(
                                        ps[:],
                                        w[:, fc, g:g + 2, :],
                                        x[:, g:g + 2, :],
                                        start=(i == 0),
                                        stop=(i == len(terms) - 1),
                                        perf_mode=DR,
                                    )
                                nc.vector.tensor_copy(
                                    dst[:, fc, t0:t0 + TT], ps[:]
                                )

                    def emit_v(x1, x2, t0):
                        w1, w2 = wres["wv1"], wres["wv2"]
                        for tc2 in range(TT // 128):
                            tsl = slice(tc2 * 128, (tc2 + 1) * 128)
                            ps = ps_v.tile([128, F], F32, tag="pv")
                            terms = (
                                [(x1, w1, g) for g in range(0, EO, 2)]
                                + [(x2, w1, g) for g in range(0, EO, 2)]
                                + [(x1, w2, g) for g in range(0, EO, 2)]
                            )
                            for i, (x, w, g) in enumerate(terms):
                                nc.tensor.matmul(
                                    ps[:],
                                    x[:, g:g + 2, tsl],
                                    w[:, g:g + 2, :],
                                    start=(i == 0),
                                    stop=(i == len(terms) - 1),
                                    perf_mode=DR,
                                )
                            # v lands at unit scale (the wv prescale is
                            # undone here) so the normalized attention
                            # output fits fp8e4's range for the hi/lo split
                            nc.vector.tensor_scalar_mul(
                                vN[:, (t0 // 128) + tc2, :], ps[:], 1.0 / WS
                            )

                    if tt == S // TT - 1:
                        # last tile: V first, so its slow PSUM->SBUF drain
                        # isn't the phase-boundary gate for the reallocated
                        # phase-2 banks
                        emit_v(x1, x2, t0)
                        emit_qk(x1, x2, t0, tt)
                    else:
                        emit_qk(x1, x2, t0, tt)
                        emit_v(x1, x2, t0)
                deferred.sort(key=lambda d: d[0] != "wk")

            # ---------- phase 2: attention per head ----------------------
            with tc.tile_pool(name="wo", bufs=1) as wo_pool:
                wo1_r = wo_pool.tile([128, HL, E], F8, tag="wo1")
                wo2_r = wo_pool.tile([128, HL, E], F8, tag="wo2")
                nc.sync.dma_start(wo1_r[:], wo1_t)
                nc.sync.dma_start(wo2_r[:], wo2_t)

                with (
                    tc.tile_pool(name="ph2", bufs=_T["EPOOL"]) as epool,
                    tc.tile_pool(name="ph2s", bufs=3) as spool,
                    tc.tile_pool(name="ph2t", bufs=2) as tpool,
                    tc.tile_pool(name="ph2b", bufs=2) as small,
                    tc.tile_pool(name="ph2f", bufs=3) as fpool,
                    tc.tile_pool(name="ps_s", bufs=_T["PSS"], space="PSUM") as ps_s,
                    tc.tile_pool(name="ps_o", bufs=2, space="PSUM") as ps_o,
                    tc.tile_pool(name="ystb", bufs=4) as ystb_pool,
                ):
                    ready_y = []

                    dq_state = {"cur": None, "idx": 0}

                    def emit_deferred_terms(nterms):
                        # deferred projection chains emitted in term-granular
                        # slices: fine-grained PE filler that matches the
                        # small per-pair Act deficit instead of overshooting
                        while nterms > 0:
                            if dq_state["cur"] is None:
                                if not deferred:
                                    return
                                wn, dst, fc, x1, x2, t0 = deferred.pop(0)
                                w1, w2 = wres[wn + "1"], wres[wn + "2"]
                                pst = ps_s.tile([128, 2, 512], F32,
                                                tag="S")
                                ps = pst[:, 0, 0:TT]
                                terms = (
                                    [(w1, x1, g) for g in range(0, EO, 2)]
                                    + [(w1, x2, g) for g in range(0, EO, 2)]
                                    + [(w2, x1, g) for g in range(0, EO, 2)]
                                )
                                dq_state["cur"] = (dst, fc, t0, ps, terms)
                                dq_state["idx"] = 0
                            dst, fc, t0, ps, terms = dq_state["cur"]
                            i0x = dq_state["idx"]
                            take = min(nterms, len(terms) - i0x)
                            for i in range(i0x, i0x + take):
                                w, x, g = terms[i]
                                nc.tensor.matmul(
                                    ps,
                                    w[:, fc, g:g + 2, :],
                                    x[:, g:g + 2, :],
                                    start=(i == 0),
                                    stop=(i == len(terms) - 1),
                                    perf_mode=DR,
                                )
                            dq_state["idx"] += take
                            nterms -= take
                            if dq_state["idx"] == len(terms):
                                nc.vector.tensor_copy(
                                    dst[:, fc, t0:t0 + TT], ps
                                )
                                dq_state["cur"] = None

                    def emit_y_pair(split_dma=False, in_ph2=True,
                                    final=False):
                        # one (tcb, et-pair) group: two Wo psum chains into a
                        # single [128, 1024] bf16 store
                        tcb, ep = ready_y.pop(0)
                        tsl = slice(tcb * 128, (tcb + 1) * 128)
                        yb = ystb_pool.tile([128, 1024], mybir.dt.bfloat16,
                                            tag="yb")
                        if in_ph2:
                            # both chains share one scores-pair tile (a
                            # half each): no dedicated y bank, so scores
                            # keep a 3-buffer rotation
                            Ypair = ps_s.tile([128, 2, 512], F32, tag="S")
                        for j in range(2):
                            esl = slice((2 * ep + j) * 512,
                                        (2 * ep + j + 1) * 512)
                            if in_ph2:
                                Ysl = (lambda jj: lambda a, b:
                                       Ypair[:, jj, a:b])(j)
                            else:
                                # phase 3: the attention O banks are free
                                Ybt = ps_o.tile([128, IT], F32, tag="O")
                                Ysl = (lambda t: lambda a, b: t[:, a:b])(Ybt)
                            Yb = Ysl(0, 512)
                            terms = []
                            for fp in range(HL // 2):
                                g = 2 * fp
                                terms += [(out1, wo1_r, g), (out2, wo1_r, g),
                                          (out1, wo2_r, g)]
                            for i, (o, w, g) in enumerate(terms):
                                nc.tensor.matmul(
                                    Yb,
                                    o[:, g:g + 2, tsl],
                                    w[:, g:g + 2, esl],
                                    start=(i == 0),
                                    stop=(i == len(terms) - 1),
                                    perf_mode=DR,
                                )
                            if final and j == 1:
                                # last store of the kernel: one Act scale,
                                # one [128,512] store -- every extra store
                                # costs a serial HWDGE slot (625ns), so no
                                # further splitting pays off
                                nc.scalar.activation(
                                    yb[:, 512:1024], Yb, COPY, scale=YS,
                                )
                                nc.sync.dma_start(y[tsl, esl],
                                                  yb[:, 512:1024])
                                continue
                            if j == 0:
                                # Act: keeps the single ps_yb bank's drain
                                # off DVE's in-order queue
                                nc.scalar.activation(
                                    yb[:, 0:512], Yb, COPY, scale=YS,
                                )
                            else:
                                # DVE (gpsimd cannot read PSUM)
                                nc.vector.tensor_scalar_mul(
                                    yb[:, 512:1024], Yb, YS
                                )
                            if split_dma or final:
                                nc.sync.dma_start(y[tsl, esl],
                                                  yb[:, j * 512:(j + 1) * 512])
                        if not (split_dma or final):
                            nc.sync.dma_start(
                                y[tsl, ep * 1024:(ep + 1) * 1024], yb[:]
                            )

                    def emit_y_quad():
                        # phase 3 only: two adjacent (tcb, ep=0/1) groups,
                        # four Wo chains, ONE [128, 2048] store -- halves the
                        # per-store issue overhead (SP config + HWDGE are a
                        # shared serial resource that otherwise can't keep
                        # pace with the chain cadence)
                        tcb, _ = ready_y.pop(0)
                        ready_y.pop(0)
                        tsl = slice(tcb * 128, (tcb + 1) * 128)
                        yb2 = ystb_pool.tile([128, 2048], mybir.dt.bfloat16,
                                             tag="yb2")
                        for half in range(4):
                            esl = slice(half * 512, (half + 1) * 512)
                            if half % 2 == 0:
                                Ybt = ps_o.tile([128, IT], F32, tag="O")
                                Yb = Ybt[:]
                            else:
                                Ybt = ps_s.tile([128, 2, 512], F32, tag="S")
                                Yb = Ybt[:, 0, :]
                            terms = []
                            for fp in range(HL // 2):
                                g = 2 * fp
                                terms += [(out1, wo1_r, g), (out2, wo1_r, g),
                                          (out1, wo2_r, g)]
                            for i, (o, w, g) in enumerate(terms):
                                nc.tensor.matmul(
                                    Yb,
                                    o[:, g:g + 2, tsl],
                                    w[:, g:g + 2, esl],
                                    start=(i == 0),
                                    stop=(i == len(terms) - 1),
                                    perf_mode=DR,
                                )
                            if half % 2 == 0:
                                nc.scalar.activation(
                                    yb2[:, esl], Yb, COPY, scale=YS,
                                )
                            else:
                                nc.vector.tensor_scalar_mul(
                                    yb2[:, esl], Yb, YS
                                )
                        nc.sync.dma_start(y[tsl, :], yb2[:])

                    deferred_split = []

                    def flush_split(on_dve=False):
                        while deferred_split:
                            Ocp_d, h_d, i0_d = deferred_split.pop(0)
                            # hi/lo split on Pool: keeps the per-head
                            # normalize chain off DVE's in-order queue.
                            # The release-critical last head of a row-block
                            # splits on DVE instead (shorter chain -> its
                            # y-pairs unlock sooner)
                            eng = nc.vector if on_dve else nc.gpsimd
                            eng.tensor_copy(
                                out1[:, h_d, i0_d:i0_d + IT], Ocp_d[:]
                            )
                            eng.tensor_sub(
                                out2[:, h_d, i0_d:i0_d + IT], Ocp_d[:],
                                out1[:, h_d, i0_d:i0_d + IT],
                            )
                            if h_d == HL - 1:
                                # the head-group's outputs are now all
                                # written -- its y pairs may be emitted
                                pd = i0_d // IT
                                for tcb_r in range(4 * pd, 4 * pd + 4):
                                    for ep_r in range(E // 1024):
                                        ready_y.append((tcb_r, ep_r))

                    # p0 (shortest, latency-bound) runs last, when y-pair
                    # chains exist to fill PE while Act/Pool/DVE drain
                    p_order = (1, 2, 0, 3)
                    for pi, p in enumerate(p_order):
                        i0 = p * IT
                        for h in range(HL):
                            h0 = h * 128
                            njc = (i0 + IT) // 128
                            npair = njc // 2
                            ndiag0 = i0 // 128   # first diag chunk index
                            O = ps_o.tile([128, IT], F32, tag="O")
                            # fp16 running rowsum accumulator; reduced
                            # across partitions at the end by one gpsimd
                            # all-reduce
                            T = tpool.tile([128, IT], F16, tag="T")

                            pending = []
                            st = {"t": False, "es": None}

                            def fill_slot(u=None, at_flush=False):
                                # PE filler: term-granular deferred chains
                                # in the Act-paced early regions, Wo chains
                                # later; p=0 fills only after its exps (so
                                # Act's in-order queue stays on exp) and
                                # keeps 2 pairs back to bridge into phase 3
                                if p == 0:
                                    if len(ready_y) > 3:
                                        emit_y_pair()
                                    if at_flush and len(ready_y) > 3:
                                        emit_y_pair()
                                elif p == 3:
                                    if (not at_flush and u in (1, npair - 1)
                                            and len(ready_y) > 4):
                                        emit_y_pair()
                                    elif at_flush and len(ready_y) > 6:
                                        emit_y_pair()
                                    elif not at_flush and u == 5:
                                        emit_deferred_terms(24)
                                elif p == 2:
                                    if at_flush and len(ready_y) > 6:
                                        emit_y_pair()
                                    elif not at_flush and u in (1, 3):
                                        emit_deferred_terms(24)
                                else:
                                    if not at_flush and u in (1, 3):
                                        emit_deferred_terms(24)

                            def emit_av(jc, Et2, i, off):
                                nc.tensor.matmul(
                                    O[:, off:],
                                    vN[:, jc, h0:h0 + 128],
                                    Et2[:, i, off:],
                                    start=(jc == 0),
                                    stop=(jc == njc - 1),
                                )

                            for u in range(npair):
                                jc0 = 2 * u
                                fill_slot(u=u)
                                # --- scores pair ---------------------------
                                ps2 = ps_s.tile([128, 2, 512], F32, tag="S")
                                Et2 = epool.tile([128, 2, 512], F16, tag="E2")
                                for i in range(2):
                                    jc = jc0 + i
                                    q_off = jc - ndiag0
                                    off = 0 if q_off < 0 else 128 * q_off
                                    nc.tensor.matmul(
                                        ps2[:, i, off:],
                                        kT[:, h, jc * 128:(jc + 1) * 128],
                                        qT[:, h, i0 + off:i0 + IT],
                                        start=True,
                                        stop=True,
                                    )
                                if jc0 + 1 < ndiag0:
                                    # non-diag pair: one exp over both banks
                                    nc.scalar.activation(
                                        Et2[:, :, :], ps2[:, :, :], EXP,
                                        scale=SCALE,
                                    )
                                    EtS = spool.tile([128, IT], F16,
                                                     tag="EtS")
                                    nc.vector.tensor_add(
                                        EtS[:], Et2[:, 0, :], Et2[:, 1, :]
                                    )
                                    if u % 2 == 1:
                                        if not st["t"]:
                                            nc.vector.tensor_add(
                                                T[:], st["es"][:], EtS[:]
                                            )
                                            st["t"] = True
                                        else:
                                            EtQ = spool.tile(
                                                [128, IT], F16, tag="EtQ")
                                            nc.vector.tensor_add(
                                                EtQ[:], st["es"][:], EtS[:]
                                            )
                                            nc.vector.tensor_add(
                                                T[:], T[:], EtQ[:]
                                            )
                                    st["es"] = EtS
                                    pending.append((jc0, Et2, 0, 0))
                                    pending.append((jc0 + 1, Et2, 1, 0))
                                else:
                                    # diag pair: ragged exps + mask + T
                                    for i in range(2):
                                        jc = jc0 + i
                                        off = 128 * (jc - ndiag0)
                                        nc.scalar.activation(
                                            Et2[:, i, off:],
                                            ps2[:, i, off:], EXP,
                                            scale=SCALE,
                                        )
                                        nc.vector.tensor_mul(
                                            Et2[:, i, off:off + 128],
                                            Et2[:, i, off:off + 128],
                                            triT[:],
                                        )
                                        if not st["t"]:
                                            nc.vector.tensor_copy(
                                                T[:], Et2[:, i, :]
                                            )
                                            st["t"] = True
                                        else:
                                            nc.vector.tensor_add(
                                                T[:, off:], T[:, off:],
                                                Et2[:, i, off:],
                                            )
                                        pending.append((jc, Et2, i, off))
                                while len(pending) > _T["PEND"]:
                                    emit_av(*pending.pop(0))
                            fill_slot(at_flush=True)
                            for item in pending:
                                emit_av(*item)
                            # rowsum across k (partitions) in one gpsimd op,
                            # broadcast to all partitions; then 1/sum on DVE
                            Rb = small.tile([128, IT], F32, tag="Rb")
                            nc.gpsimd.partition_all_reduce(
                                Rb[:], T[:], channels=128,
                                reduce_op=bass_isa.ReduceOp.add,
                            )
                            rec = small.tile([128, IT], F32, tag="rec")
                            nc.vector.reciprocal(rec[:], Rb[:])
                            Ocp = fpool.tile([128, IT], F32, tag="Ocp")
                            nc.vector.tensor_mul(Ocp[:], O[:], rec[:])
                            # defer the fp8 hi/lo split of this iteration's
                            # output until the next iteration, so Pool's
                            # all-reduce is never queued behind a hi-copy;
                            # the last head of each row-block flushes
                            # immediately so its y-pairs unlock before the
                            # next region's filler slots
                            flush_split()
                            deferred_split.append((Ocp, h, i0))
                            if h == HL - 1:
                                flush_split(on_dve=True)

                    # ---- phase 3: remaining output-projection groups ------
                    # batch stores two-pairs-at-a-time; the last two pairs
                    # split/quarter their stores so the post-PE drain is
                    # short
                    while ready_y:
                        if (len(ready_y) > 4
                                and ready_y[0][0] == ready_y[1][0]
                                and ready_y[0][1] == 0
                                and ready_y[1][1] == 1):
                            emit_y_quad()
                        else:
                            emit_y_pair(split_dma=False, in_ph2=False,
                                        final=(len(ready_y) == 1))
            xhold.release()
            whold.release()
    nc.compile()
    return nc


def _get_nc():
    if "nc" not in _CACHE:
        _CACHE["nc"] = _build()
    return _CACHE["nc"]


def _split8(a):
    hi = a.astype(NPF8)
    lo = (a - hi.astype(np.float32)).astype(NPF8)
    return hi, lo


def make_in_maps(x, Wq, Wk, Wv, Wo):
    x = np.asarray(x, np.float32)
    Wq = np.asarray(Wq, np.float32)
    Wk = np.asarray(Wk, np.float32)
    Wv = np.asarray(Wv, np.float32)
    Wo = np.asarray(Wo, np.float32)

    # keep-mask in [k_local (partition), q_local (col)] layout: keep k<=q
    tri01 = np.ascontiguousarray(np.triu(np.ones((128, 128), np.float16)))

    def _pack_x(xT8):
        # [E, S] -> [ei=128, tt, eo, TT] (phase-1 tile DMA order)
        return np.ascontiguousarray(
            xT8.reshape(EO, 128, S // TT, TT).transpose(1, 2, 0, 3)
        )

    xs = []
    for b in range(B):
        hi, lo = _split8(np.ascontiguousarray(x[b].T))
        xs.append((_pack_x(hi), _pack_x(lo)))
    def _pack_qk(w8):  # [E, F] -> [ei=128, fc, eo, 128]
        return np.ascontiguousarray(
            w8.reshape(EO, 128, HL, 128).transpose(1, 2, 0, 3)
        )

    def _pack_v(w8):  # [E, F] -> [ei=128, eo, F]
        return np.ascontiguousarray(
            w8.reshape(EO, 128, F).transpose(1, 0, 2)
        )

    def _pack_o(w8):  # [F, E] -> [fi=128, fc, E]
        return np.ascontiguousarray(
            w8.reshape(HL, 128, E).transpose(1, 0, 2)
        )

    in_maps = []
    for c in range(8):
        b, g = c // 4, c % 4
        fsl = slice(F * g, F * (g + 1))
        m = {"x1P": xs[b][0], "x2P": xs[b][1], "tri01": tri01}
        for n, W, pk in (("wq", Wq, _pack_qk), ("wk", Wk, _pack_qk),
                         ("wv", Wv, _pack_v)):
            hi, lo = _split8(np.ascontiguousarray(W[fsl, :].T) * WS)
            m[n + "1"], m[n + "2"] = pk(hi), pk(lo)
        hi, lo = _split8(np.ascontiguousarray(Wo[:, fsl].T) * WS)
        m["wo1"], m["wo2"] = _pack_o(hi), _pack_o(lo)
        in_maps.append(m)
    return in_maps


def combine_outputs(results):
    out = np.empty((B, S, E), np.float32)
    for b in range(B):
        acc = results[4 * b]["y"].astype(np.float32).copy()
        for g in range(1, 4):
            acc += results[4 * b + g]["y"]
        out[b] = acc
    return out


def kernel(x, Wq, Wk, Wv, Wo):
    import time as _time

    nc = _get_nc()
    in_maps = make_in_maps(x, Wq, Wk, Wv, Wo)
    last_exc = None
    for attempt in range(3):
        if attempt:
            # transient device wedge (e.g. NRT_EXEC_UNIT_UNRECOVERABLE) --
            # wait for recovery before retrying
            _time.sleep(30 * attempt)
        try:
            res = bass_utils.run_bass_kernel_spmd(
                nc, in_maps, core_ids=list(range(8))
            )
            return combine_outputs(res.results)
        except Exception as exc:
            last_exc = exc
    raise last_exc
